# revision 25
# baseline (speedup 1.0000x reference)
"""Distributed GQA attention prefill kernel for 8 Trainium2 NeuronCores.

Sharding: query rows interleaved with stride 8 (core c owns positions
c, c+8, c+16, ... of each batch; 512 local rows), weights replicated.
Each core computes its local Q/K/V projections + RoPE, the RoPE'd K^T and V
shards are AllGathered in one packed collective, each core runs causal
attention for its rows against the causal prefix of K/V, then applies the
output projection.  The output is row-sharded (strided) -> host gather.

Causal load balance: with stride-8 interleaving, every 128-row q-tile t
spans positions [1024t, 1024(t+1)), so it needs 8t full 128-key chunks plus
one 1024-wide "stepped diagonal" band, identical on every core (uniform
SPMD graph); the step pattern depends only on the core id and enters as a
data mask.  Keys are staged band-major so each band is contiguous.

Precision modes (selected at runtime from input statistics):
  - fp16 (score sigma small, e.g. the 0.02-init regime): single-term fp16
    matmuls everywhere (11-bit mantissa factors), multiplicative 0/1
    causal mask on the exp output (gpsimd), renormalized sums.
  - hp (large score sigma, argmax-like softmax): split-bf16 3-term matmuls
    for q/k projections and QK^T (~17-bit factors), additive -1e9 mask on
    scores in PSUM.
Both modes: v projection, attn@V and output projection in 16-bit 1-term.

A legacy path handles arbitrary (non-causal) masks via exp(mask) multiply.
"""

import math
import sys
import types

sys.path.insert(0, "/opt/trn_rl_repo")

if "antenv.axon_hooks" not in sys.modules:
    _m = types.ModuleType("antenv.axon_hooks")
    _m.get_axon_ntff_profile_hook = lambda: None
    sys.modules["antenv.axon_hooks"] = _m

import numpy as np
import ml_dtypes

import concourse.bass as bass
import concourse.tile as tile
from concourse import bacc, mybir
from concourse.bass_utils import run_bass_kernel_spmd

B, S, D = 2, 2048, 4096
H, KVH, HD = 32, 8, 128
NREP = H // KVH
N_CORES = 8
SL = S // N_CORES          # 256 positions per core per batch
LR = B * SL                # 512 local query rows per core
P = 128
F32 = mybir.dt.float32
BF16 = mybir.dt.bfloat16
F16 = mybir.dt.float16
KVC = KVH * HD             # 1024 kv cols
KCH = D // P               # 32 contraction chunks
NKT = S // P               # 16 key chunks of 128

TIMING_R = 0   # >0: wrap body in For_i(R), replace collective with local DMA
               # -1: single body, collective replaced (for CoreSim analysis)

_GRAPH_CACHE = {}
_LAST_IN_MAPS = None


# --------------------------------------------------------------------------
# graph construction
# --------------------------------------------------------------------------

def _build_graph(mode):
    """mode: 'c16', 'chp', 'n16', 'nhp', 'emask'."""
    nc = bacc.Bacc(None, target_bir_lowering=False, debug=False,
                   num_devices=N_CORES)
    causal = mode[0] == "c"
    fp16 = mode.endswith("16")
    DT = F16 if fp16 else BF16

    t = {}
    if fp16:
        t["xt"] = nc.declare_dram_parameter("xt", [P, KCH, LR], DT, False)
        t["wq_f"] = nc.declare_dram_parameter("wq_f", [H, P, KCH, P], DT, False)
        t["wk_f"] = nc.declare_dram_parameter("wk_f", [KVH, P, KCH, P], DT, False)
    else:
        t["xt_hi"] = nc.declare_dram_parameter("xt_hi", [P, KCH, LR], BF16, False)
        t["xt_lo"] = nc.declare_dram_parameter("xt_lo", [P, KCH, LR], BF16, False)
        t["wq_hi"] = nc.declare_dram_parameter("wq_hi", [H, P, KCH, P], BF16, False)
        t["wq_lo"] = nc.declare_dram_parameter("wq_lo", [H, P, KCH, P], BF16, False)
        t["wk_hi"] = nc.declare_dram_parameter("wk_hi", [KVH, P, KCH, P], BF16, False)
        t["wk_lo"] = nc.declare_dram_parameter("wk_lo", [KVH, P, KCH, P], BF16, False)
    t["wv_b"] = nc.declare_dram_parameter("wv_b", [D, KVC], DT, False)
    t["wo_b"] = nc.declare_dram_parameter("wo_b", [D // 512, P, KCH, 512], BF16, False)
    if fp16:
        t["cbias"] = nc.declare_dram_parameter("cbias", [P, 1], F32, False)
    if causal:
        if fp16:
            t["bmask"] = nc.declare_dram_parameter("bmask", [P, 1024], BF16, False)
        else:
            t["bmask"] = nc.declare_dram_parameter("bmask", [P, 1024], F32, False)
    if mode == "emask":
        t["emask"] = nc.declare_dram_parameter("emask", [SL, S], F32, False)
    t["cosT"] = nc.declare_dram_parameter("cosT", [HD, SL], F32, False)
    t["sinT"] = nc.declare_dram_parameter("sinT", [HD, SL], F32, False)
    t["cosTu"] = nc.declare_dram_parameter("cosTu", [HD, SL], F32, False)
    t["sinTu"] = nc.declare_dram_parameter("sinTu", [HD, SL], F32, False)
    t["perm"] = nc.declare_dram_parameter("perm", [P, P], F32, False)
    t["out_ext"] = nc.declare_dram_parameter("out", [LR, D], F32, True)

    with tile.TileContext(nc) as tc:
        emit = _emit_emask if mode == "emask" else (
            lambda a, b, c: _emit_main(a, b, c, causal, fp16))
        if TIMING_R > 0:
            with tc.For_i(0, TIMING_R, 1):
                emit(nc, tc, t)
        else:
            emit(nc, tc, t)
    nc.compile()
    return nc


def _rope_out(nc, pool, ps_pool, psum_in, perm_t, cos_t, sin_t, outs, uid):
    """PSUM [128, LR] fp32 projection -> RoPE (transposed layout: even/odd
    partition pairs rotated via a partition-swapped local DMA) -> write to
    outs (one DT ap, or (hi, lo) bf16 aps for the split-precision path)."""
    qT = pool.tile([P, LR], F32, tag="ropeT", name=f"qT{uid}")
    nc.scalar.copy(qT[:], psum_in[:])
    psw = pool.tile([P, LR], F32, tag="ropeS", name=f"psw{uid}")
    qT_v = qT[:].rearrange("(a t) r -> t a r", t=2)
    psw_v = psw[:].rearrange("(a t) r -> t a r", t=2)
    nc.scalar.dma_start(psw_v[0], qT_v[1])
    nc.scalar.dma_start(psw_v[1], qT_v[0])
    tmp = pool.tile([P, LR], F32, tag="ropeU", name=f"tmp{uid}")
    swp = pool.tile([P, LR], F32, tag="ropeV", name=f"swp{uid}")
    for b in range(B):
        bsl = slice(b * SL, (b + 1) * SL)
        nc.vector.tensor_mul(tmp[:, bsl], qT[:, bsl], cos_t[:])
        nc.vector.tensor_mul(swp[:, bsl], psw[:, bsl], sin_t[:])
    if len(outs) == 1:
        nc.vector.tensor_add(outs[0], tmp[:], swp[:])
    else:
        hi_out, lo_out = outs
        rot = pool.tile([P, LR], F32, tag="ropeW", name=f"rot{uid}")
        nc.vector.tensor_add(rot[:], tmp[:], swp[:])
        nc.scalar.copy(hi_out, rot[:])
        nc.vector.tensor_sub(lo_out, rot[:], hi_out)


def _emit_main(nc, tc, t, causal, fp16):
    from contextlib import ExitStack
    from concourse.masks import make_identity
    out_ext = t["out_ext"]
    DT = F16 if fp16 else BF16
    # packed AG payload in f32 columns: K^T [hi] (+lo if hp) | V
    KPACK = 512            # 1024 DT = 512 f32 cols
    PACK = (2 if fp16 else 3) * KPACK
    LOFF = 1024            # DT-col offset of K lo (hp)
    VOFF = 1024 if fp16 else 2048   # DT-col offset of V

    with ExitStack() as ctx:
        const = ctx.enter_context(tc.tile_pool(name="const", bufs=1))
        qsp_pool = ctx.enter_context(tc.tile_pool(name="qsp_pool", bufs=1))
        dram = ctx.enter_context(tc.tile_pool(name="dram", bufs=1, space="DRAM"))

        kv_loc = dram.tile([LR, PACK], F32)
        kv_full = dram.tile([N_CORES * LR, PACK], F32, addr_space="Shared")

        ident_b = const.tile([P, P], BF16)
        make_identity(nc, ident_b)
        cb_t = None
        if fp16:
            cb_t = const.tile([P, 1], F32)
            nc.sync.dma_start(cb_t[:], t["cbias"].ap()[:, :])
        if causal:
            bmask_t = const.tile([P, 1024], BF16 if fp16 else F32)
            nc.sync.dma_start(bmask_t[:], t["bmask"].ap()[:, :])

        qh = qsp_pool.tile([P, H, LR], DT)
        ql = None if fp16 else qsp_pool.tile([P, H, LR], BF16)

        # ---------------- phase A: projections ----------------
        with ExitStack() as actx:
            ac = actx.enter_context(tc.tile_pool(name="ac", bufs=1))
            xt_pool = actx.enter_context(tc.tile_pool(name="xt_pool", bufs=1))

            xh = xt_pool.tile([P, KCH, LR], DT)
            xl = None if fp16 else xt_pool.tile([P, KCH, LR], BF16)
            if fp16:
                xh_src = t["xt"].ap()
            else:
                xh_src = t["xt_hi"].ap()
                xl_src = t["xt_lo"].ap()
            for q4 in range(4):
                ksl4 = slice(q4 * (KCH // 4), (q4 + 1) * (KCH // 4))
                eng = nc.sync if q4 % 2 == 0 else nc.scalar
                eng.dma_start(xh[:, ksl4, :], xh_src[:, ksl4, :])
                if not fp16:
                    eng.dma_start(xl[:, ksl4, :], xl_src[:, ksl4, :])

            cosT_t = ac.tile([P, SL], F32)
            sinT_t = ac.tile([P, SL], F32)
            cosTu_t = ac.tile([P, SL], F32)
            sinTu_t = ac.tile([P, SL], F32)
            nc.scalar.dma_start(cosT_t[:], t["cosT"].ap()[:, :])
            nc.scalar.dma_start(sinT_t[:], t["sinT"].ap()[:, :])
            nc.scalar.dma_start(cosTu_t[:], t["cosTu"].ap()[:, :])
            nc.scalar.dma_start(sinTu_t[:], t["sinTu"].ap()[:, :])
            perm_t = ac.tile([P, P], F32)
            nc.scalar.dma_start(perm_t[:], t["perm"].ap()[:, :])

            # ---- k projection -> K^T, RoPE, pack ----
            with ExitStack() as kctx:
                wkp = kctx.enter_context(tc.tile_pool(name="wkp", bufs=6))
                kev = kctx.enter_context(tc.tile_pool(name="kev", bufs=2))
                ppk = kctx.enter_context(
                    tc.tile_pool(name="ppk", bufs=2, space="PSUM"))
                ppw = kctx.enter_context(
                    tc.tile_pool(name="ppw", bufs=2, space="PSUM"))
                for g in range(KVH):
                    if fp16:
                        wkh = wkp.tile([P, KCH, P], DT, tag="wk", name=f"wkh{g}")
                        for q2 in range(2):
                            k2 = slice(q2 * (KCH // 2), (q2 + 1) * (KCH // 2))
                            nc.sync.dma_start(
                                wkh[:, k2, :],
                                t["wk_f"].ap()[g][:, k2, :])
                    else:
                        wkh = wkp.tile([P, KCH, P], BF16, tag="wk", name=f"wkh{g}")
                        wkl = wkp.tile([P, KCH, P], BF16, tag="wk", name=f"wkl{g}")
                        for q2 in range(2):
                            k2 = slice(q2 * (KCH // 2), (q2 + 1) * (KCH // 2))
                            nc.sync.dma_start(
                                wkh[:, k2, :],
                                t["wk_hi"].ap()[g][:, k2, :])
                            nc.sync.dma_start(
                                wkl[:, k2, :],
                                t["wk_lo"].ap()[g][:, k2, :])
                    ps = ppk.tile([P, LR], F32, tag="pk", name=f"pk{g}")
                    for ck in range(KCH):
                        if fp16:
                            nc.tensor.matmul(ps[:], wkh[:, ck, :], xh[:, ck, :],
                                             start=(ck == 0), stop=(ck == KCH - 1))
                        else:
                            nc.tensor.matmul(ps[:], wkh[:, ck, :], xh[:, ck, :],
                                             start=(ck == 0), stop=False)
                            nc.tensor.matmul(ps[:], wkh[:, ck, :], xl[:, ck, :],
                                             start=False, stop=False)
                            nc.tensor.matmul(ps[:], wkl[:, ck, :], xh[:, ck, :],
                                             start=False, stop=(ck == KCH - 1))
                    khs = kev.tile([P, LR], DT, tag="khx", name=f"khx{g}")
                    kls = None if fp16 else kev.tile([P, LR], BF16, tag="klx",
                                                     name=f"klx{g}")
                    _rope_out(nc, kev, ppw, ps, perm_t, cosTu_t, sinTu_t,
                              [khs[:]] if fp16 else [khs[:], kls[:]],
                              uid=f"k{g}")
                    # pack rows sub*128+p; f32 cols [g*64,(g+1)*64) (+lo at 512)
                    dst = kv_loc[:, :].rearrange("(sub p) c -> p sub c", p=P)
                    src_h = khs[:].rearrange(
                        "p (sub c) -> p sub c", sub=LR // P).bitcast(F32)
                    nc.sync.dma_start(dst[:, :, g * 64:(g + 1) * 64], src_h)
                    if not fp16:
                        src_l = kls[:].rearrange(
                            "p (sub c) -> p sub c", sub=LR // P).bitcast(F32)
                        nc.sync.dma_start(
                            dst[:, :, 512 + g * 64:512 + (g + 1) * 64], src_l)

            # ---- v projection (natural layout) ----
            with ExitStack() as vctx:
                wvs = vctx.enter_context(tc.tile_pool(name="wvs", bufs=8))
                vev = vctx.enter_context(tc.tile_pool(name="vev", bufs=3))
                ppv = vctx.enter_context(
                    tc.tile_pool(name="ppv", bufs=4, space="PSUM"))
                for cg in range(KVC // 512):             # 2 col groups of 512
                    csl = slice(cg * 512, (cg + 1) * 512)
                    pv = [ppv.tile([P, 512], F32, tag="pv",
                                   name=f"pv{cg}_{i}") for i in range(4)]
                    for ck in range(KCH):
                        wvt = wvs.tile([P, 512], DT, tag="wvt",
                                       name=f"wvt{cg}_{ck}")
                        nc.sync.dma_start(
                            wvt[:], t["wv_b"].ap()[ck * P:(ck + 1) * P, csl])
                        for rt in range(4):
                            rsl = slice(rt * P, (rt + 1) * P)
                            nc.tensor.matmul(pv[rt][:], xh[:, ck, rsl], wvt[:],
                                             start=(ck == 0),
                                             stop=(ck == KCH - 1))
                    for rt in range(4):
                        ve = vev.tile([P, 512], BF16, tag="ve",
                                      name=f"ve{cg}_{rt}")
                        nc.scalar.copy(ve[:], pv[rt][:])
                        nc.sync.dma_start(
                            kv_loc[rt * P:(rt + 1) * P,
                                   VOFF // 2 + cg * 256:VOFF // 2 + (cg + 1) * 256],
                            ve[:].bitcast(F32))

            # ---- AllGather of packed K^T | V ----
            if TIMING_R != 0:
                nc.scalar.dma_start(kv_full[0:LR, :], kv_loc[:, :])
            else:
                nc.gpsimd.collective_compute(
                    "AllGather", mybir.AluOpType.bypass,
                    replica_groups=[list(range(N_CORES))],
                    ins=[kv_loc.opt()],
                    outs=[kv_full.opt()],
                )

            # ---- q projection + RoPE ----
            with ExitStack() as qctx:
                wqp = qctx.enter_context(tc.tile_pool(name="wqp", bufs=5))
                qev = qctx.enter_context(tc.tile_pool(name="qev", bufs=2))
                ppq = qctx.enter_context(
                    tc.tile_pool(name="ppq", bufs=2, space="PSUM"))
                ppw2 = qctx.enter_context(
                    tc.tile_pool(name="ppw2", bufs=2, space="PSUM"))
                for h in range(H):
                    if fp16:
                        wqh = wqp.tile([P, KCH, P], DT, tag="wq", name=f"wqh{h}")
                        for q2 in range(2):
                            k2 = slice(q2 * (KCH // 2), (q2 + 1) * (KCH // 2))
                            nc.sync.dma_start(
                                wqh[:, k2, :],
                                t["wq_f"].ap()[h][:, k2, :])
                    else:
                        wqh = wqp.tile([P, KCH, P], BF16, tag="wq", name=f"wqh{h}")
                        wql = wqp.tile([P, KCH, P], BF16, tag="wq", name=f"wql{h}")
                        for q2 in range(2):
                            k2 = slice(q2 * (KCH // 2), (q2 + 1) * (KCH // 2))
                            nc.sync.dma_start(
                                wqh[:, k2, :],
                                t["wq_hi"].ap()[h][:, k2, :])
                            nc.sync.dma_start(
                                wql[:, k2, :],
                                t["wq_lo"].ap()[h][:, k2, :])
                    ps = ppq.tile([P, LR], F32, tag="pq", name=f"pq{h}")
                    for ck in range(KCH):
                        if fp16:
                            nc.tensor.matmul(ps[:], wqh[:, ck, :], xh[:, ck, :],
                                             start=(ck == 0), stop=(ck == KCH - 1))
                        else:
                            nc.tensor.matmul(ps[:], wqh[:, ck, :], xh[:, ck, :],
                                             start=(ck == 0), stop=False)
                            nc.tensor.matmul(ps[:], wqh[:, ck, :], xl[:, ck, :],
                                             start=False, stop=False)
                            nc.tensor.matmul(ps[:], wql[:, ck, :], xh[:, ck, :],
                                             start=False, stop=(ck == KCH - 1))
                    _rope_out(nc, qev, ppw2, ps, perm_t, cosT_t, sinT_t,
                              [qh[:, h, :]] if fp16 else [qh[:, h, :], ql[:, h, :]],
                              uid=f"q{h}")

        # ---------------- phase B: attention ----------------
        aoT_pool = ctx.enter_context(tc.tile_pool(name="aoT_pool", bufs=1))
        aoT = aoT_pool.tile([P, H, LR], BF16)

        _emit_attention(nc, tc, t, kv_full, qh, ql, aoT, ident_b,
                        bmask_t if causal else None, cb_t, causal, fp16,
                        KPACK, LOFF, VOFF)

        # ---------------- phase C: output projection ----------------
        with ExitStack() as cctx:
            wop = cctx.enter_context(tc.tile_pool(name="wop", bufs=2))
            osb = cctx.enter_context(tc.tile_pool(name="osb", bufs=3))
            ps_o = cctx.enter_context(
                tc.tile_pool(name="ps_o", bufs=4, space="PSUM"))
            for dg in range(D // 512):  # 8
                dsl = slice(dg * 512, (dg + 1) * 512)
                wot = wop.tile([P, KCH, 512], BF16, tag="wo", name=f"wo{dg}")
                wo_src = t["wo_b"].ap()[dg]
                for q4 in range(4):
                    ksl4 = slice(q4 * (KCH // 4), (q4 + 1) * (KCH // 4))
                    nc.sync.dma_start(wot[:, ksl4, :], wo_src[:, ksl4, :])
                for rt in range(4):
                    rsl = slice(rt * P, (rt + 1) * P)
                    ps = ps_o.tile([P, 512], F32, tag="po",
                                   name=f"po{dg}_{rt}")
                    for ck in range(KCH):
                        nc.tensor.matmul(ps[:], aoT[:, ck, rsl], wot[:, ck, :],
                                         start=(ck == 0), stop=(ck == KCH - 1))
                    ot = osb.tile([P, 512], F32, tag="ot", name=f"ot{dg}_{rt}")
                    nc.scalar.copy(ot[:], ps[:])
                    nc.sync.dma_start(out_ext.ap()[rt * P:(rt + 1) * P, dsl], ot[:])


def _emit_attention(nc, tc, t, kv_full, qh, ql, aoT, ident_b, bmask_t,
                    cb_t, causal, fp16, KPACK, LOFF, VOFF):
    from contextlib import ExitStack
    DT = F16 if fp16 else BF16

    with ExitStack() as bctx:
        kst = bctx.enter_context(tc.tile_pool(name="kst", bufs=3))
        vst = bctx.enter_context(tc.tile_pool(name="vst", bufs=3))
        scp = bctx.enter_context(tc.tile_pool(name="scp", bufs=3))
        atp = bctx.enter_context(tc.tile_pool(name="atp", bufs=2))
        sml = bctx.enter_context(tc.tile_pool(name="sml", bufs=8))
        ps_sc = bctx.enter_context(
            tc.tile_pool(name="ps_sc", bufs=2, space="PSUM"))
        ps_tr = bctx.enter_context(
            tc.tile_pool(name="ps_tr", bufs=2, space="PSUM"))
        ps_av = bctx.enter_context(
            tc.tile_pool(name="ps_av", bufs=2, space="PSUM"))

        kvb = kv_full[:, :].bitcast(DT)
        src = kvb.rearrange(
            "(r e hj p) c -> p r e hj c", p=P, e=B, hj=SL // P)
        srcv = kv_full[:, :].bitcast(BF16).rearrange(
            "(r e hj p) c -> p r e hj c", p=P, e=B, hj=SL // P)

        def softmax_group(psc, eraw_sl, nmax_ap, rsum_ap, masked, uid):
            """psc [P,1024] raw scores -> eraw = exp(psc + bias); accumulates
            rsum.  fp16 mode: constant bias (scores are small), masked via 0/1
            multiply; hp mode: additive -1e9 mask, per-group row max."""
            if masked and not fp16:
                pass  # multiplicative mask below
            if not fp16:
                if masked:
                    nc.vector.tensor_add(psc[:], psc[:], bmask_t[:])
                nc.vector.tensor_reduce(
                    nmax_ap, psc[:], axis=mybir.AxisListType.XY,
                    op=mybir.AluOpType.max, negate=True)
                bias = nmax_ap
            else:
                bias = cb_t[:]
            nc.scalar.activation(
                eraw_sl, psc[:], mybir.ActivationFunctionType.Exp,
                bias=bias, scale=1.0, accum_out=rsum_ap)
            if masked and fp16:
                nc.vector.scalar_tensor_tensor(
                    out=eraw_sl, in0=eraw_sl, scalar=1.0, in1=bmask_t[:],
                    op0=mybir.AluOpType.bypass, op1=mybir.AluOpType.mult,
                    accum_out=rsum_ap)

        for b in range(B):
            for g in range(KVH):
                # stage K^T (+lo) and V, band-major chunk order (hj*8+r)
                kh_s = kst.tile([P, NKT, P], DT, tag="khs", name=f"khs{b}_{g}")
                kl_s = None if fp16 else kst.tile([P, NKT, P], BF16, tag="kls",
                                                  name=f"kls{b}_{g}")
                vn = vst.tile([P, NKT, HD], BF16, tag="vn",
                               name=f"vn{b}_{g}")
                kh_v = kh_s[:].rearrange("p (hj r) c -> p hj r c", hj=SL // P)
                vn_v = vn[:].rearrange("p (hj r) c -> p hj r c", hj=SL // P)
                for hj in range(SL // P):
                    nc.sync.dma_start(
                        kh_v[:, hj, :, :],
                        src[:, :, b, hj, g * P:(g + 1) * P])
                    nc.sync.dma_start(
                        vn_v[:, hj, :, :],
                        srcv[:, :, b, hj, VOFF + g * P:VOFF + (g + 1) * P])
                    if not fp16:
                        kl_v = kl_s[:].rearrange("p (hj r) c -> p hj r c",
                                                 hj=SL // P)
                        nc.sync.dma_start(
                            kl_v[:, hj, :, :],
                            src[:, :, b, hj, LOFF + g * P:LOFF + (g + 1) * P])
                kh_m = kh_s[:].rearrange("p a c -> p (a c)")
                kl_m = None if fp16 else kl_s[:].rearrange("p a c -> p (a c)")

                for r in range(NREP):
                    h = g * NREP + r
                    u0 = f"{b}_{g}_{r}"
                    aT = atp.tile([P, NKT, 2 * P], BF16, tag="aT",
                                  name=f"aT{u0}")
                    eraw0 = scp.tile([P, 2048], BF16, tag="er0",
                                     name=f"er0{u0}")
                    eraw1 = scp.tile([P, 2048], BF16, tag="er1",
                                     name=f"er1{u0}")

                    def qk_matmuls(psc, qrsl, cols):
                        # cols: slice of kh_m DT columns (multiple of 512)
                        n512 = (cols.stop - cols.start) // 512
                        for kt in range(n512):
                            ksl = slice(cols.start + kt * 512,
                                        cols.start + (kt + 1) * 512)
                            osl = slice(kt * 512, (kt + 1) * 512)
                            if fp16:
                                nc.tensor.matmul(
                                    psc[:, osl], qh[:, h, qrsl], kh_m[:, ksl],
                                    start=True, stop=True)
                            else:
                                nc.tensor.matmul(
                                    psc[:, osl], qh[:, h, qrsl], kh_m[:, ksl],
                                    start=True, stop=False)
                                nc.tensor.matmul(
                                    psc[:, osl], ql[:, h, qrsl], kh_m[:, ksl],
                                    start=False, stop=False)
                                nc.tensor.matmul(
                                    psc[:, osl], qh[:, h, qrsl], kl_m[:, ksl],
                                    start=False, stop=True)

                    def softmax_tile(qrsl, eraw, halves, uid):
                        """halves: list of (col_slice, masked).  Writes
                        UNNORMALIZED exp into eraw; returns per-half diag
                        normalizer matrices (folded into the PE transpose)."""
                        nh = len(halves)
                        rowmax = sml.tile([P, nh], F32, tag="rmax",
                                          name=f"rm{uid}")
                        rsum = sml.tile([P, nh], F32, tag="rsum",
                                        name=f"rs{uid}")
                        for i, (csl, masked) in enumerate(halves):
                            psc = ps_sc.tile([P, 1024], F32, tag="psc",
                                             name=f"psc{uid}_{i}")
                            qk_matmuls(psc, qrsl, csl)
                            softmax_group(
                                psc, eraw[:, csl.start:csl.stop],
                                rowmax[:, i:i + 1], rsum[:, i:i + 1],
                                masked, uid=f"{uid}_{i}")
                        recip = sml.tile([P, 1], F32, tag="recip",
                                         name=f"rc{uid}")
                        if fp16 or nh == 1:
                            # common exp bias across halves -> plain sum
                            if nh == 1:
                                nc.vector.reciprocal(recip[:], rsum[:])
                            else:
                                tots = sml.tile([P, 1], F32, tag="tots",
                                                name=f"to{uid}")
                                nc.vector.tensor_reduce(
                                    tots[:], rsum[:],
                                    axis=mybir.AxisListType.XY,
                                    op=mybir.AluOpType.add)
                                nc.vector.reciprocal(recip[:], tots[:])
                            wd = sml.tile([P, P], BF16, tag="wd",
                                          name=f"wd{uid}")
                            nc.vector.tensor_scalar_mul(wd[:], ident_b[:],
                                                        recip[:])
                            return [wd] * nh
                        # hp: per-half max -> fixup scales th_h/Z
                        negmax = sml.tile([P, 1], F32, tag="nmax",
                                          name=f"nx{uid}")
                        nc.vector.tensor_reduce(
                            negmax[:], rowmax[:], axis=mybir.AxisListType.XY,
                            op=mybir.AluOpType.min)
                        th = sml.tile([P, nh], F32, tag="th", name=f"th{uid}")
                        for i in range(nh):
                            nc.scalar.activation(
                                th[:, i:i + 1], rowmax[:, i:i + 1],
                                mybir.ActivationFunctionType.Exp,
                                bias=negmax[:], scale=-1.0)
                        prod = sml.tile([P, nh], F32, tag="prod",
                                        name=f"pr{uid}")
                        tots = sml.tile([P, 1], F32, tag="tots",
                                        name=f"to{uid}")
                        nc.vector.tensor_mul(prod[:], rsum[:], th[:])
                        nc.vector.tensor_reduce(
                            tots[:], prod[:], axis=mybir.AxisListType.XY,
                            op=mybir.AluOpType.add)
                        nc.vector.reciprocal(recip[:], tots[:])
                        sc2 = sml.tile([P, nh], F32, tag="sc2",
                                       name=f"sc{uid}")
                        nc.vector.tensor_scalar_mul(sc2[:], th[:], recip[:])
                        wds = []
                        for i in range(nh):
                            wd = sml.tile([P, P], BF16, tag="wd",
                                          name=f"wd{uid}_{i}")
                            nc.vector.tensor_scalar_mul(wd[:], ident_b[:],
                                                        sc2[:, i:i + 1])
                            wds.append(wd)
                        return wds

                    q0sl = slice(b * SL, b * SL + P)
                    q1sl = slice(b * SL + P, b * SL + 2 * P)
                    if causal:
                        halves0 = [(slice(0, 1024), True)]
                        halves1 = [(slice(0, 1024), False),
                                   (slice(1024, 2048), True)]
                        nj0 = 8
                    else:
                        halves0 = [(slice(0, 1024), False),
                                   (slice(1024, 2048), False)]
                        halves1 = halves0
                        nj0 = NKT
                    wds0 = softmax_tile(q0sl, eraw0, halves0, f"a{u0}")
                    wds1 = softmax_tile(q1sl, eraw1, halves1, f"b{u0}")

                    # ---- transpose attn -> aT (normalization folded into
                    # the transpose weight: diag(scale) instead of identity)
                    tb_i = 0
                    for src_er, nj, tcol, wds in ((eraw0, nj0, 0, wds0),
                                                  (eraw1, NKT, 1, wds1)):
                        for tb in range(nj // 4):
                            pst = ps_tr.tile([P, 512], F32, tag="ptr",
                                             name=f"ptr{u0}_{tcol}_{tb}")
                            for jj in range(4):
                                j = tb * 4 + jj
                                wd = wds[min(j // 8, len(wds) - 1)]
                                nc.tensor.matmul(
                                    pst[:, jj * P:(jj + 1) * P],
                                    src_er[:, j * P:(j + 1) * P],
                                    wd[:], start=True, stop=True)
                            dst = aT[:, tb * 4:(tb + 1) * 4,
                                     tcol * P:(tcol + 1) * P]
                            srcp = pst[:].rearrange("p (a c) -> p a c", a=4)
                            if tb_i % 2 == 0:
                                nc.vector.tensor_copy(dst, srcp)
                            else:
                                nc.scalar.copy(dst, srcp)
                            tb_i += 1

                    # ---- attn @ V ----
                    povw = 384 if causal else 256
                    pov = ps_av.tile([P, povw], F32, tag="pov",
                                     name=f"pov{u0}")
                    pov01 = pov[:, 0:256]
                    for j in range(nj0):
                        nc.tensor.matmul(pov01, vn[:, j, :], aT[:, j, :],
                                         start=(j == 0), stop=(j == nj0 - 1))
                    if causal:
                        pov1b = pov[:, 256:384]
                        for j in range(8, 16):
                            nc.tensor.matmul(pov1b, vn[:, j, :],
                                             aT[:, j, P:2 * P],
                                             start=(j == 8), stop=(j == 15))
                        nc.scalar.copy(aoT[:, h, b * SL:b * SL + P],
                                       pov[:, 0:P])
                        p1tmp = sml.tile([P, P], F32, tag="p1t",
                                         name=f"p1t{u0}")
                        nc.scalar.copy(p1tmp[:], pov[:, P:2 * P])
                        nc.vector.tensor_add(
                            aoT[:, h, b * SL + P:b * SL + 2 * P],
                            pov1b, p1tmp[:])
                    else:
                        nc.scalar.copy(aoT[:, h, b * SL:(b + 1) * SL],
                                       pov01)


def _emit_emask(nc, tc, t):
    """Legacy path: arbitrary mask via exp(mask) multiply, full attention,
    split-bf16 precision.  (Structure of the original baseline kernel.)"""
    from contextlib import ExitStack
    from concourse.masks import make_identity
    out_ext = t["out_ext"]
    PACK = 1536
    with ExitStack() as ctx:
        const = ctx.enter_context(tc.tile_pool(name="const", bufs=1))
        qsp_pool = ctx.enter_context(tc.tile_pool(name="qsp_pool", bufs=1))
        dram = ctx.enter_context(tc.tile_pool(name="dram", bufs=1, space="DRAM"))

        kv_loc = dram.tile([LR, PACK], F32)
        kv_full = dram.tile([N_CORES * LR, PACK], F32, addr_space="Shared")

        ident_b = const.tile([P, P], BF16)
        make_identity(nc, ident_b)

        qh = qsp_pool.tile([P, H, LR], BF16)
        ql = qsp_pool.tile([P, H, LR], BF16)

        with ExitStack() as actx:
            ac = actx.enter_context(tc.tile_pool(name="ac", bufs=1))
            xt_pool = actx.enter_context(tc.tile_pool(name="xt_pool", bufs=1))

            cosT_t = ac.tile([P, SL], F32)
            sinT_t = ac.tile([P, SL], F32)
            cosTu_t = ac.tile([P, SL], F32)
            sinTu_t = ac.tile([P, SL], F32)
            nc.sync.dma_start(cosT_t[:], t["cosT"].ap()[:, :])
            nc.sync.dma_start(sinT_t[:], t["sinT"].ap()[:, :])
            nc.sync.dma_start(cosTu_t[:], t["cosTu"].ap()[:, :])
            nc.sync.dma_start(sinTu_t[:], t["sinTu"].ap()[:, :])
            perm_t = ac.tile([P, P], F32)
            nc.sync.dma_start(perm_t[:], t["perm"].ap()[:, :])

            xh = xt_pool.tile([P, KCH, LR], BF16)
            xl = xt_pool.tile([P, KCH, LR], BF16)
            xh_src = t["xt_hi"].ap()
            xl_src = t["xt_lo"].ap()
            for q4 in range(4):
                ksl4 = slice(q4 * (KCH // 4), (q4 + 1) * (KCH // 4))
                nc.sync.dma_start(xh[:, ksl4, :], xh_src[:, ksl4, :])
                nc.sync.dma_start(xl[:, ksl4, :], xl_src[:, ksl4, :])

            with ExitStack() as kctx:
                wkp = kctx.enter_context(tc.tile_pool(name="wkp", bufs=6))
                kev = kctx.enter_context(tc.tile_pool(name="kev", bufs=2))
                ppk = kctx.enter_context(
                    tc.tile_pool(name="ppk", bufs=2, space="PSUM"))
                ppw = kctx.enter_context(
                    tc.tile_pool(name="ppw", bufs=2, space="PSUM"))
                for g in range(KVH):
                    wkh = wkp.tile([P, KCH, P], BF16, tag="wk", name=f"wkh{g}")
                    wkl = wkp.tile([P, KCH, P], BF16, tag="wk", name=f"wkl{g}")
                    for q2 in range(2):
                        k2 = slice(q2 * (KCH // 2), (q2 + 1) * (KCH // 2))
                        nc.sync.dma_start(
                            wkh[:, k2, :],
                            t["wk_hi"].ap()[g][:, k2, :])
                        nc.sync.dma_start(
                            wkl[:, k2, :],
                            t["wk_lo"].ap()[g][:, k2, :])
                    ps = ppk.tile([P, LR], F32, tag="pk", name=f"pk{g}")
                    for ck in range(KCH):
                        nc.tensor.matmul(ps[:], wkh[:, ck, :], xh[:, ck, :],
                                         start=(ck == 0), stop=False)
                        nc.tensor.matmul(ps[:], wkh[:, ck, :], xl[:, ck, :],
                                         start=False, stop=False)
                        nc.tensor.matmul(ps[:], wkl[:, ck, :], xh[:, ck, :],
                                         start=False, stop=(ck == KCH - 1))
                    khs = kev.tile([P, LR], BF16, tag="khx", name=f"khx{g}")
                    kls = kev.tile([P, LR], BF16, tag="klx", name=f"klx{g}")
                    _rope_out(nc, kev, ppw, ps, perm_t, cosTu_t, sinTu_t,
                              [khs[:], kls[:]], uid=f"k{g}")
                    src_h = khs[:].rearrange(
                        "p (sub c) -> p sub c", sub=LR // P).bitcast(F32)
                    src_l = kls[:].rearrange(
                        "p (sub c) -> p sub c", sub=LR // P).bitcast(F32)
                    dst = kv_loc[:, :].rearrange("(sub p) c -> p sub c", p=P)
                    nc.sync.dma_start(dst[:, :, g * 64:(g + 1) * 64], src_h)
                    nc.sync.dma_start(
                        dst[:, :, 512 + g * 64:512 + (g + 1) * 64], src_l)

            with ExitStack() as vctx:
                wvs = vctx.enter_context(tc.tile_pool(name="wvs", bufs=8))
                vev = vctx.enter_context(tc.tile_pool(name="vev", bufs=3))
                ppv = vctx.enter_context(
                    tc.tile_pool(name="ppv", bufs=4, space="PSUM"))
                for cg in range(KVC // 512):
                    csl = slice(cg * 512, (cg + 1) * 512)
                    pv = [ppv.tile([P, 512], F32, tag="pv",
                                   name=f"pv{cg}_{i}") for i in range(4)]
                    for ck in range(KCH):
                        wvt = wvs.tile([P, 512], BF16, tag="wvt",
                                       name=f"wvt{cg}_{ck}")
                        nc.sync.dma_start(
                            wvt[:], t["wv_b"].ap()[ck * P:(ck + 1) * P, csl])
                        for rt in range(4):
                            rsl = slice(rt * P, (rt + 1) * P)
                            nc.tensor.matmul(pv[rt][:], xh[:, ck, rsl], wvt[:],
                                             start=(ck == 0),
                                             stop=(ck == KCH - 1))
                    for rt in range(4):
                        ve = vev.tile([P, 512], BF16, tag="ve",
                                      name=f"ve{cg}_{rt}")
                        nc.scalar.copy(ve[:], pv[rt][:])
                        nc.sync.dma_start(
                            kv_loc[rt * P:(rt + 1) * P,
                                   1024 + cg * 256:1024 + (cg + 1) * 256],
                            ve[:].bitcast(F32))

            if TIMING_R != 0:
                nc.scalar.dma_start(kv_full[0:LR, :], kv_loc[:, :])
            else:
                nc.gpsimd.collective_compute(
                    "AllGather", mybir.AluOpType.bypass,
                    replica_groups=[list(range(N_CORES))],
                    ins=[kv_loc.opt()],
                    outs=[kv_full.opt()],
                )

            with ExitStack() as qctx:
                wqp = qctx.enter_context(tc.tile_pool(name="wqp", bufs=5))
                qev = qctx.enter_context(tc.tile_pool(name="qev", bufs=2))
                ppq = qctx.enter_context(
                    tc.tile_pool(name="ppq", bufs=2, space="PSUM"))
                ppw2 = qctx.enter_context(
                    tc.tile_pool(name="ppw2", bufs=2, space="PSUM"))
                for h in range(H):
                    wqh = wqp.tile([P, KCH, P], BF16, tag="wq", name=f"wqh{h}")
                    wql = wqp.tile([P, KCH, P], BF16, tag="wq", name=f"wql{h}")
                    for q2 in range(2):
                        k2 = slice(q2 * (KCH // 2), (q2 + 1) * (KCH // 2))
                        nc.sync.dma_start(
                            wqh[:, k2, :],
                            t["wq_hi"].ap()[h][:, k2, :])
                        nc.sync.dma_start(
                            wql[:, k2, :],
                            t["wq_lo"].ap()[h][:, k2, :])
                    ps = ppq.tile([P, LR], F32, tag="pq", name=f"pq{h}")
                    for ck in range(KCH):
                        nc.tensor.matmul(ps[:], wqh[:, ck, :], xh[:, ck, :],
                                         start=(ck == 0), stop=False)
                        nc.tensor.matmul(ps[:], wqh[:, ck, :], xl[:, ck, :],
                                         start=False, stop=False)
                        nc.tensor.matmul(ps[:], wql[:, ck, :], xh[:, ck, :],
                                         start=False, stop=(ck == KCH - 1))
                    _rope_out(nc, qev, ppw2, ps, perm_t, cosT_t, sinT_t,
                              [qh[:, h, :], ql[:, h, :]], uid=f"q{h}")

        aoT_pool = ctx.enter_context(tc.tile_pool(name="aoT_pool", bufs=1))
        aoT = aoT_pool.tile([P, H, LR], BF16)

        with ExitStack() as bctx:
            bc = bctx.enter_context(tc.tile_pool(name="bc", bufs=1))
            kst = bctx.enter_context(tc.tile_pool(name="kst", bufs=3))
            vst = bctx.enter_context(tc.tile_pool(name="vst", bufs=3))
            scp = bctx.enter_context(tc.tile_pool(name="scp", bufs=3))
            atp = bctx.enter_context(tc.tile_pool(name="atp", bufs=3))
            sml = bctx.enter_context(tc.tile_pool(name="sml", bufs=8))
            ps_sc = bctx.enter_context(
                tc.tile_pool(name="ps_sc", bufs=2, space="PSUM"))
            ps_tr = bctx.enter_context(
                tc.tile_pool(name="ps_tr", bufs=2, space="PSUM"))
            ps_av = bctx.enter_context(
                tc.tile_pool(name="ps_av", bufs=2, space="PSUM"))

            mask_t = bc.tile([P, 2, S], F32)
            nc.sync.dma_start(
                mask_t[:], t["emask"].ap().rearrange("(a p) c -> p a c", p=P))

            kvb = kv_full[:, :].bitcast(BF16)
            src = kvb.rearrange(
                "(r e hj p) c -> p r e hj c", p=P, e=B, hj=SL // P)
            for b in range(B):
                for g in range(KVH):
                    kh_s = kst.tile([P, NKT, P], BF16, tag="khs",
                                    name=f"khs{b}_{g}")
                    kl_s = kst.tile([P, NKT, P], BF16, tag="kls",
                                    name=f"kls{b}_{g}")
                    vn = vst.tile([P, NKT, HD], BF16, tag="vn",
                                  name=f"vn{b}_{g}")
                    for hj in range(SL // P):
                        kh_v = kh_s[:].rearrange("p (r hj) c -> p r hj c",
                                                 hj=SL // P)
                        kl_v = kl_s[:].rearrange("p (r hj) c -> p r hj c",
                                                 hj=SL // P)
                        vnv = vn[:].rearrange("p (r hj) c -> p r hj c",
                                              hj=SL // P)
                        nc.sync.dma_start(
                            kh_v[:, :, hj, :],
                            src[:, :, b, hj, g * P:(g + 1) * P])
                        nc.sync.dma_start(
                            kl_v[:, :, hj, :],
                            src[:, :, b, hj, 1024 + g * P:1024 + (g + 1) * P])
                        nc.sync.dma_start(
                            vnv[:, :, hj, :],
                            src[:, :, b, hj, 2048 + g * P:2048 + (g + 1) * P])
                    kh_m = kh_s[:].rearrange("p a c -> p (a c)")
                    kl_m = kl_s[:].rearrange("p a c -> p (a c)")

                    for r in range(NREP):
                        h = g * NREP + r
                        aT = atp.tile([P, NKT, SL], BF16, tag="aT",
                                      name=f"aT{b}_{g}_{r}")
                        for qs in range(SL // P):
                            u = f"{b}_{g}_{r}_{qs}"
                            qrsl = slice(b * SL + qs * P,
                                         b * SL + (qs + 1) * P)
                            eraw = scp.tile([P, S], BF16, tag="eraw",
                                            name=f"eraw{u}")
                            rowmax = sml.tile([P, 2], F32, tag="rmax",
                                              name=f"rmax{u}")
                            rsum = sml.tile([P, 2], F32, tag="rsum",
                                            name=f"rsum{u}")
                            negmax = sml.tile([P, 1], F32, tag="nmax",
                                              name=f"nmax{u}")
                            for half in range(2):
                                psc = ps_sc.tile([P, 1024], F32, tag="psc",
                                                 name=f"psc{u}_{half}")
                                for kt in range(2):
                                    ksl = slice((half * 2 + kt) * 512,
                                                (half * 2 + kt + 1) * 512)
                                    osl = slice(kt * 512, (kt + 1) * 512)
                                    nc.tensor.matmul(
                                        psc[:, osl], qh[:, h, qrsl],
                                        kh_m[:, ksl], start=True, stop=False)
                                    nc.tensor.matmul(
                                        psc[:, osl], ql[:, h, qrsl],
                                        kh_m[:, ksl], start=False, stop=False)
                                    nc.tensor.matmul(
                                        psc[:, osl], qh[:, h, qrsl],
                                        kl_m[:, ksl], start=False, stop=True)
                                nc.vector.tensor_reduce(
                                    rowmax[:, half:half + 1], psc[:],
                                    axis=mybir.AxisListType.XY,
                                    op=mybir.AluOpType.max, negate=True)
                                nc.scalar.activation(
                                    eraw[:, half * 1024:(half + 1) * 1024],
                                    psc[:],
                                    mybir.ActivationFunctionType.Exp,
                                    bias=rowmax[:, half:half + 1], scale=1.0,
                                    accum_out=rsum[:, half:half + 1])
                            nc.vector.tensor_reduce(
                                negmax[:], rowmax[:],
                                axis=mybir.AxisListType.XY,
                                op=mybir.AluOpType.min)
                            th = sml.tile([P, 2], F32, tag="th", name=f"th{u}")
                            for half in range(2):
                                nc.scalar.activation(
                                    th[:, half:half + 1],
                                    rowmax[:, half:half + 1],
                                    mybir.ActivationFunctionType.Exp,
                                    bias=negmax[:], scale=-1.0)
                            tots = sml.tile([P, 1], F32, tag="tots",
                                            name=f"tots{u}")
                            prod = sml.tile([P, 2], F32, tag="prod",
                                            name=f"prod{u}")
                            nc.vector.tensor_mul(prod[:], rsum[:], th[:])
                            nc.vector.scalar_tensor_tensor(
                                out=eraw[:], in0=eraw[:], scalar=1.0,
                                in1=mask_t[:, qs, :],
                                op0=mybir.AluOpType.bypass,
                                op1=mybir.AluOpType.mult)
                            for half in range(2):
                                hsl = slice(half * 1024, (half + 1) * 1024)
                                nc.vector.tensor_reduce(
                                    rsum[:, half:half + 1], eraw[:, hsl],
                                    axis=mybir.AxisListType.XY,
                                    op=mybir.AluOpType.add)
                            nc.vector.tensor_mul(prod[:], rsum[:], th[:])
                            nc.vector.tensor_reduce(
                                tots[:], prod[:],
                                axis=mybir.AxisListType.XY,
                                op=mybir.AluOpType.add)
                            recip = sml.tile([P, 1], F32, tag="recip",
                                             name=f"recip{u}")
                            nc.vector.reciprocal(recip[:], tots[:])
                            sc2 = sml.tile([P, 2], F32, tag="sc2",
                                           name=f"sc2{u}")
                            nc.vector.tensor_scalar_mul(sc2[:], th[:],
                                                        recip[:])
                            for half in range(2):
                                hsl = slice(half * 1024, (half + 1) * 1024)
                                nc.vector.tensor_scalar_mul(
                                    eraw[:, hsl], eraw[:, hsl],
                                    sc2[:, half:half + 1])
                            for tb in range(4):
                                pst = ps_tr.tile([P, 512], BF16, tag="ptr",
                                                 name=f"ptr{u}_{tb}")
                                for j in range(4):
                                    jj = tb * 4 + j
                                    nc.tensor.transpose(
                                        pst[:, j * P:(j + 1) * P],
                                        eraw[:, jj * P:(jj + 1) * P],
                                        ident_b)
                                dst = aT[:, tb * 4:(tb + 1) * 4,
                                         qs * P:(qs + 1) * P]
                                srcp = pst[:].rearrange(
                                    "p (a c) -> p a c", a=4)
                                if (tb + qs) % 2 == 0:
                                    nc.vector.tensor_copy(dst, srcp)
                                else:
                                    nc.scalar.copy(dst, srcp)
                        pov = ps_av.tile([P, SL], F32, tag="pov",
                                         name=f"pov{b}_{g}_{r}")
                        for j in range(NKT):
                            nc.tensor.matmul(pov[:], vn[:, j, :], aT[:, j, :],
                                             start=(j == 0),
                                             stop=(j == NKT - 1))
                        nc.scalar.copy(aoT[:, h, b * SL:(b + 1) * SL], pov[:])

        with ExitStack() as cctx:
            wop = cctx.enter_context(tc.tile_pool(name="wop", bufs=2))
            osb = cctx.enter_context(tc.tile_pool(name="osb", bufs=3))
            ps_o = cctx.enter_context(
                tc.tile_pool(name="ps_o", bufs=4, space="PSUM"))
            for dg in range(D // 512):
                dsl = slice(dg * 512, (dg + 1) * 512)
                wot = wop.tile([P, KCH, 512], BF16, tag="wo", name=f"wo{dg}")
                wo_src = t["wo_b"].ap()[dg]
                for q4 in range(4):
                    ksl4 = slice(q4 * (KCH // 4), (q4 + 1) * (KCH // 4))
                    nc.sync.dma_start(wot[:, ksl4, :], wo_src[:, ksl4, :])
                for rt in range(4):
                    rsl = slice(rt * P, (rt + 1) * P)
                    ps = ps_o.tile([P, 512], F32, tag="po",
                                   name=f"po{dg}_{rt}")
                    for ck in range(KCH):
                        nc.tensor.matmul(ps[:], aoT[:, ck, rsl], wot[:, ck, :],
                                         start=(ck == 0), stop=(ck == KCH - 1))
                    ot = osb.tile([P, 512], F32, tag="ot", name=f"ot{dg}_{rt}")
                    nc.scalar.copy(ot[:], ps[:])
                    nc.sync.dma_start(out_ext.ap()[rt * P:(rt + 1) * P, dsl],
                                      ot[:])


# --------------------------------------------------------------------------
# host side
# --------------------------------------------------------------------------

def _split_bf16(a):
    hi = a.astype(ml_dtypes.bfloat16)
    lo = (a - hi.astype(np.float32)).astype(ml_dtypes.bfloat16)
    return hi, lo


def _tile_w(w, nh):
    # [D, nh*HD] -> [nh, P, KCH, P] with [p, k, c] = w[k*P+p, h*HD+c]
    return np.ascontiguousarray(
        w.reshape(KCH, P, nh, P).transpose(2, 1, 0, 3))


def _tile_wo(wo):
    # [H*HD, D] -> [8, P, KCH, 512] with [dg, p, k, c] = wo[k*P+p, dg*512+c]
    return np.ascontiguousarray(
        wo.reshape(KCH, P, 8, 512).transpose(2, 1, 0, 3))


def _tile_x(xt):
    # [D, LR] -> [P, KCH, LR]
    return np.ascontiguousarray(xt.reshape(KCH, P, LR).transpose(1, 0, 2))


def _rows_slice(mode, c):
    # strided rows for the main paths, contiguous for the legacy path
    if mode == "emask":
        return slice(c * SL, (c + 1) * SL)
    return slice(c, S, N_CORES)


def _host_prep(mode, x, wq, wk, wv, wo, freqs_cos, freqs_sin, mask):
    fp16 = mode.endswith("16")
    causal = mode[0] == "c"
    scale = 1.0 / np.sqrt(HD)

    if fp16:
        wq_f = _tile_w(wq, H).astype(np.float16)
        wk_f = _tile_w(wk, KVH).astype(np.float16)
        wv_b = wv.astype(np.float16)
        wo_b = _tile_wo(wo.astype(ml_dtypes.bfloat16))
        # constant exp bias: scores are bounded by ~6 sigma in this regime
        sx = float(x.std())
        sig = (sx * float(wq.std()) * math.sqrt(D)) * \
              (sx * float(wk.std()) * math.sqrt(D))
        cb = np.full((P, 1), -(6.0 * sig + 2.0), np.float32)
    else:
        wq_hi, wq_lo = (_tile_w(a, H) for a in _split_bf16(wq))
        wk_hi, wk_lo = (_tile_w(a, KVH) for a in _split_bf16(wk))
        wv_b = wv.astype(ml_dtypes.bfloat16)
        wo_b = _tile_wo(wo.astype(ml_dtypes.bfloat16))

    perm = np.zeros((P, P), np.float32)
    idx = np.arange(P)
    perm[idx, idx ^ 1] = 1.0  # pair swap

    in_maps = []
    for c in range(N_CORES):
        sl = _rows_slice(mode, c)
        x_loc = np.concatenate([x[0, sl], x[1, sl]], axis=0)  # [LR, D]
        xt = np.ascontiguousarray(x_loc.T)                    # [D, LR]

        fc = freqs_cos[sl]  # [SL, HD//2]
        fs = freqs_sin[sl]
        # transposed layout: freq i on partitions 2i/2i+1; sin sign: -s on
        # even rows, +s on odd rows.  q version carries the 1/sqrt(HD) scale.
        cosTu = np.repeat(fc.T, 2, axis=0)                    # [HD, SL]
        sinTu = np.repeat(fs.T, 2, axis=0).copy()
        sinTu[0::2] *= -1.0
        cosT = cosTu * scale
        sinT = sinTu * scale

        m = {
            "wv_b": wv_b, "wo_b": wo_b,
            "cosT": np.ascontiguousarray(cosT),
            "sinT": np.ascontiguousarray(sinT),
            "cosTu": np.ascontiguousarray(cosTu),
            "sinTu": np.ascontiguousarray(sinTu),
            "perm": perm,
        }
        if fp16:
            m["xt"] = _tile_x(xt.astype(np.float16))
            m["wq_f"] = wq_f
            m["wk_f"] = wk_f
            m["cbias"] = cb
        else:
            xt_hi, xt_lo = _split_bf16(xt)
            m["xt_hi"] = _tile_x(xt_hi)
            m["xt_lo"] = _tile_x(xt_lo)
            m["wq_hi"], m["wq_lo"] = wq_hi, wq_lo
            m["wk_hi"], m["wk_lo"] = wk_hi, wk_lo
        if causal:
            # band mask in staged coordinates: col rblk*128+ik <-> position
            # offset 8*ik+rblk within the band; row i <-> offset c+8*i.
            ik = np.arange(P)
            qoff = (c + 8 * ik)[:, None]                      # [128,1]
            rblk, kk = np.meshgrid(np.arange(8), np.arange(P), indexing="ij")
            koff = (8 * kk + rblk).reshape(-1)[None, :]       # [1,1024]
            allowed = koff <= qoff                            # [128,1024]
            if fp16:
                m["bmask"] = allowed.astype(ml_dtypes.bfloat16)
            else:
                m["bmask"] = np.where(allowed, 0.0, -1e9).astype(np.float32)
        if mode == "emask":
            mask_loc = np.exp(np.ascontiguousarray(
                np.broadcast_to(mask[0, 0], (S, S))[sl]))     # exp(mask)
            m["emask"] = mask_loc.astype(np.float32)
        in_maps.append(m)
    return in_maps


def _select_mode(x, wq, wk, wv, wo, mask):
    # causal / no-mask / general-mask
    if not np.any(mask != 0.0):
        masktype = "n"
    else:
        m2 = np.broadcast_to(mask[0, 0], (S, S))
        tril = np.tril(np.ones((S, S), bool))
        if np.all(m2[tril] == 0.0) and np.all(m2[~tril] <= -1e8):
            masktype = "c"
        else:
            return "emask"
    # precision: single-term fp16 suffices when softmax is diffuse
    sx = float(x.std())
    sq = sx * float(wq.std()) * math.sqrt(D)
    sk = sx * float(wk.std()) * math.sqrt(D)
    sig_score = sq * sk
    maxabs = max(float(np.abs(a).max()) for a in (x, wq, wk, wv, wo))
    fp16_ok = sig_score < 8.0 and maxabs < 2000.0 and sq < 500 and sk < 500
    return masktype + ("16" if fp16_ok else "hp")


def kernel(x, wq, wk, wv, wo, freqs_cos, freqs_sin, mask, start_pos=0, **_):
    x = np.asarray(x, dtype=np.float32)
    wq = np.asarray(wq, dtype=np.float32)
    wk = np.asarray(wk, dtype=np.float32)
    wv = np.asarray(wv, dtype=np.float32)
    wo = np.asarray(wo, dtype=np.float32)
    freqs_cos = np.asarray(freqs_cos, dtype=np.float32)
    freqs_sin = np.asarray(freqs_sin, dtype=np.float32)
    mask = np.asarray(mask, dtype=np.float32)

    mode = _select_mode(x, wq, wk, wv, wo, mask)
    if mode not in _GRAPH_CACHE:
        _GRAPH_CACHE[mode] = _build_graph(mode)
    nc = _GRAPH_CACHE[mode]

    in_maps = _host_prep(mode, x, wq, wk, wv, wo, freqs_cos, freqs_sin, mask)
    global _LAST_IN_MAPS
    _LAST_IN_MAPS = in_maps
    _GRAPH_CACHE["last_nc"] = nc
    _GRAPH_CACHE["last_mode"] = mode

    res = run_bass_kernel_spmd(nc, in_maps, core_ids=list(range(N_CORES)))

    out = np.empty((B, S, D), np.float32)
    for c in range(N_CORES):
        o = res.results[c]["out"]  # [LR, D]
        sl = _rows_slice(mode, c)
        out[0, sl] = o[:SL]
        out[1, sl] = o[SL:]
    return out


if __name__ == "__main__":
    rng = np.random.default_rng(0)
    inputs = {
        "x": rng.standard_normal((B, S, D), dtype=np.float32),
        "wq": rng.standard_normal((D, H * HD), dtype=np.float32) * 0.02,
        "wk": rng.standard_normal((D, KVC), dtype=np.float32) * 0.02,
        "wv": rng.standard_normal((D, KVC), dtype=np.float32) * 0.02,
        "wo": rng.standard_normal((H * HD, D), dtype=np.float32) * 0.02,
        "freqs_cos": rng.random((S, HD // 2), dtype=np.float32),
        "freqs_sin": rng.random((S, HD // 2), dtype=np.float32),
        "mask": np.zeros((1, 1, S, S), np.float32),
        "start_pos": 0,
    }
    out = kernel(**inputs)
    print("kernel output:", out.shape, out.dtype)


# revision 36
# speedup vs baseline: 1.0552x; 1.0552x over previous
"""Distributed GQA attention prefill kernel for 8 Trainium2 NeuronCores.

Sharding: query rows interleaved with stride 8 (core c owns positions
c, c+8, c+16, ... of each batch; 512 local rows), weights replicated.
Each core computes its local Q/K/V projections + RoPE, the RoPE'd K^T and V
shards are AllGathered in one packed collective, each core runs causal
attention for its rows against the causal prefix of K/V, then applies the
output projection.  The output is row-sharded (strided) -> host gather.

Causal load balance: with stride-8 interleaving, every 128-row q-tile t
spans positions [1024t, 1024(t+1)), so it needs 8t full 128-key chunks plus
one 1024-wide "stepped diagonal" band, identical on every core (uniform
SPMD graph); the step pattern depends only on the core id and enters as a
data mask.  Keys are staged band-major so each band is contiguous.

Precision modes (selected at runtime from input statistics):
  - fp16 (score sigma small, e.g. the 0.02-init regime): single-term fp16
    matmuls for q/k projections and QK^T (11-bit mantissa factors).
    Softmax skips the row-max pass: exp(score - C) against a host-chosen
    constant bound C, probabilities kept UNNORMALIZED in bf16 (wide
    exponent), causal mask as a 0/1 multiply fused with the row-sum
    (scalar_tensor_tensor accum), and the 1/Z normalization is folded into
    the attn transpose by using diag(1/Z) instead of the identity as the
    PE transpose weight (zero extra vector work).
  - hp (large score sigma, argmax-like softmax): split-bf16 3-term matmuls
    for q/k projections and QK^T (~17-bit factors), additive -1e9 mask on
    scores in PSUM, per-group row max with cross-group fixup; fixup scales
    likewise folded into diag transpose weights.
Both modes: v projection, attn@V and output projection in 16-bit 1-term.
RoPE pair-swap is a partition-swapped local DMA (no PE work).  Weights are
host-retiled partition-major so every weight DMA is one descriptor per
partition.  No gpsimd compute (HW gpsimd op dispatch costs ~10us/op).

A legacy path handles arbitrary (non-causal) masks via exp(mask) multiply.
"""

import math
import sys
import types

sys.path.insert(0, "/opt/trn_rl_repo")

if "antenv.axon_hooks" not in sys.modules:
    _m = types.ModuleType("antenv.axon_hooks")
    _m.get_axon_ntff_profile_hook = lambda: None
    sys.modules["antenv.axon_hooks"] = _m

import numpy as np
import ml_dtypes

import concourse.bass as bass
import concourse.tile as tile
from concourse import bacc, mybir
from concourse.bass_utils import run_bass_kernel_spmd

B, S, D = 2, 2048, 4096
H, KVH, HD = 32, 8, 128
NREP = H // KVH
N_CORES = 8
SL = S // N_CORES          # 256 positions per core per batch
LR = B * SL                # 512 local query rows per core
P = 128
F32 = mybir.dt.float32
BF16 = mybir.dt.bfloat16
F16 = mybir.dt.float16
KVC = KVH * HD             # 1024 kv cols
KCH = D // P               # 32 contraction chunks
NKT = S // P               # 16 key chunks of 128

TIMING_R = 0   # >0: wrap body in For_i(R), replace collective with local DMA
               # -1: single body, collective replaced (for CoreSim analysis)

_GRAPH_CACHE = {}
_LAST_IN_MAPS = None


# --------------------------------------------------------------------------
# graph construction
# --------------------------------------------------------------------------

def _build_graph(mode):
    """mode: 'c16', 'chp', 'n16', 'nhp', 'emask'."""
    nc = bacc.Bacc(None, target_bir_lowering=False, debug=False,
                   num_devices=N_CORES)
    causal = mode[0] == "c"
    fp16 = mode.endswith("16")
    DT = F16 if fp16 else BF16

    t = {}
    if fp16:
        t["xt"] = nc.declare_dram_parameter("xt", [P, KCH, LR], DT, False)
        t["wq_f"] = nc.declare_dram_parameter("wq_f", [H, P, KCH, P], DT, False)
        t["wk_f"] = nc.declare_dram_parameter("wk_f", [KVH, P, KCH, P], DT, False)
    else:
        t["xt_hi"] = nc.declare_dram_parameter("xt_hi", [P, KCH, LR], BF16, False)
        t["xt_lo"] = nc.declare_dram_parameter("xt_lo", [P, KCH, LR], BF16, False)
        t["wq_hi"] = nc.declare_dram_parameter("wq_hi", [H, P, KCH, P], BF16, False)
        t["wq_lo"] = nc.declare_dram_parameter("wq_lo", [H, P, KCH, P], BF16, False)
        t["wk_hi"] = nc.declare_dram_parameter("wk_hi", [KVH, P, KCH, P], BF16, False)
        t["wk_lo"] = nc.declare_dram_parameter("wk_lo", [KVH, P, KCH, P], BF16, False)
    t["wv_b"] = nc.declare_dram_parameter("wv_b", [D, KVC], DT, False)
    t["wo_b"] = nc.declare_dram_parameter("wo_b", [D // 512, P, KCH, 512], BF16, False)
    if fp16:
        t["cbias"] = nc.declare_dram_parameter("cbias", [P, 1], F32, False)
    if causal:
        if fp16:
            t["bmask"] = nc.declare_dram_parameter("bmask", [P, 1024], BF16, False)
        else:
            t["bmask"] = nc.declare_dram_parameter("bmask", [P, 1024], F32, False)
    if mode == "emask":
        t["emask"] = nc.declare_dram_parameter("emask", [SL, S], F32, False)
    t["cosT"] = nc.declare_dram_parameter("cosT", [HD, SL], F32, False)
    t["sinT"] = nc.declare_dram_parameter("sinT", [HD, SL], F32, False)
    t["cosTu"] = nc.declare_dram_parameter("cosTu", [HD, SL], F32, False)
    t["sinTu"] = nc.declare_dram_parameter("sinTu", [HD, SL], F32, False)
    t["perm"] = nc.declare_dram_parameter("perm", [P, P], F32, False)
    t["out_ext"] = nc.declare_dram_parameter("out", [LR, D], F32, True)

    with tile.TileContext(nc) as tc:
        emit = _emit_emask if mode == "emask" else (
            lambda a, b, c: _emit_main(a, b, c, causal, fp16))
        if TIMING_R > 0:
            with tc.For_i(0, TIMING_R, 1):
                emit(nc, tc, t)
        else:
            emit(nc, tc, t)
    nc.compile()
    return nc


def _rope_out(nc, pool, ps_pool, psum_in, perm_t, cos_t, sin_t, outs, uid):
    """PSUM [128, LR] fp32 projection -> RoPE (transposed layout: even/odd
    partition pairs rotated via a partition-swapped local DMA) -> write to
    outs (one DT ap, or (hi, lo) bf16 aps for the split-precision path)."""
    qT = pool.tile([P, LR], F32, tag="ropeT", name=f"qT{uid}")
    nc.scalar.copy(qT[:], psum_in[:])
    psw = pool.tile([P, LR], F32, tag="ropeS", name=f"psw{uid}")
    qT_v = qT[:].rearrange("(a t) r -> t a r", t=2)
    psw_v = psw[:].rearrange("(a t) r -> t a r", t=2)
    nc.scalar.dma_start(psw_v[0], qT_v[1])
    nc.scalar.dma_start(psw_v[1], qT_v[0])
    tmp = pool.tile([P, LR], F32, tag="ropeU", name=f"tmp{uid}")
    swp = pool.tile([P, LR], F32, tag="ropeV", name=f"swp{uid}")
    for b in range(B):
        bsl = slice(b * SL, (b + 1) * SL)
        nc.vector.tensor_mul(tmp[:, bsl], qT[:, bsl], cos_t[:])
        nc.vector.tensor_mul(swp[:, bsl], psw[:, bsl], sin_t[:])
    if len(outs) == 1:
        nc.vector.tensor_add(outs[0], tmp[:], swp[:])
    else:
        hi_out, lo_out = outs
        rot = pool.tile([P, LR], F32, tag="ropeW", name=f"rot{uid}")
        nc.vector.tensor_add(rot[:], tmp[:], swp[:])
        nc.scalar.copy(hi_out, rot[:])
        nc.vector.tensor_sub(lo_out, rot[:], hi_out)


def _emit_main(nc, tc, t, causal, fp16):
    from contextlib import ExitStack
    from concourse.masks import make_identity
    out_ext = t["out_ext"]
    DT = F16 if fp16 else BF16
    # packed AG payload in f32 columns: K^T [hi] (+lo if hp) | V
    KPACK = 512            # 1024 DT = 512 f32 cols
    PACK = (2 if fp16 else 3) * KPACK
    LOFF = 1024            # DT-col offset of K lo (hp)
    VOFF = 1024 if fp16 else 2048   # DT-col offset of V

    with ExitStack() as ctx:
        const = ctx.enter_context(tc.tile_pool(name="const", bufs=1))
        qsp_pool = ctx.enter_context(tc.tile_pool(name="qsp_pool", bufs=1))
        dram = ctx.enter_context(tc.tile_pool(name="dram", bufs=1, space="DRAM"))

        kv_loc = dram.tile([LR, PACK], F32)
        kv_full = dram.tile([N_CORES * LR, PACK], F32, addr_space="Shared")

        ident_b = const.tile([P, P], BF16)
        make_identity(nc, ident_b)
        cb_t = None
        if fp16:
            cb_t = const.tile([P, 1], F32)
            nc.sync.dma_start(cb_t[:], t["cbias"].ap()[:, :])
        if causal:
            bmask_t = const.tile([P, 1024], BF16 if fp16 else F32)
            nc.sync.dma_start(bmask_t[:], t["bmask"].ap()[:, :])

        qh = qsp_pool.tile([P, H, LR], DT)
        ql = None if fp16 else qsp_pool.tile([P, H, LR], BF16)

        # ---------------- phase A: projections ----------------
        with ExitStack() as actx:
            ac = actx.enter_context(tc.tile_pool(name="ac", bufs=1))
            xt_pool = actx.enter_context(tc.tile_pool(name="xt_pool", bufs=1))

            xh = xt_pool.tile([P, KCH, LR], DT)
            xl = None if fp16 else xt_pool.tile([P, KCH, LR], BF16)
            if fp16:
                xh_src = t["xt"].ap()
            else:
                xh_src = t["xt_hi"].ap()
                xl_src = t["xt_lo"].ap()
            for q4 in range(4):
                ksl4 = slice(q4 * (KCH // 4), (q4 + 1) * (KCH // 4))
                eng = nc.sync if q4 % 2 == 0 else nc.scalar
                eng.dma_start(xh[:, ksl4, :], xh_src[:, ksl4, :])
                if not fp16:
                    eng.dma_start(xl[:, ksl4, :], xl_src[:, ksl4, :])

            cosT_t = ac.tile([P, SL], F32)
            sinT_t = ac.tile([P, SL], F32)
            cosTu_t = ac.tile([P, SL], F32)
            sinTu_t = ac.tile([P, SL], F32)
            nc.scalar.dma_start(cosT_t[:], t["cosT"].ap()[:, :])
            nc.scalar.dma_start(sinT_t[:], t["sinT"].ap()[:, :])
            nc.scalar.dma_start(cosTu_t[:], t["cosTu"].ap()[:, :])
            nc.scalar.dma_start(sinTu_t[:], t["sinTu"].ap()[:, :])
            perm_t = ac.tile([P, P], F32)
            nc.scalar.dma_start(perm_t[:], t["perm"].ap()[:, :])

            # ---- k projection -> K^T, RoPE, pack ----
            with ExitStack() as kctx:
                wkp = kctx.enter_context(tc.tile_pool(name="wkp", bufs=5))
                kev = kctx.enter_context(tc.tile_pool(name="kev", bufs=2))
                ppk = kctx.enter_context(
                    tc.tile_pool(name="ppk", bufs=2, space="PSUM"))
                ppw = kctx.enter_context(
                    tc.tile_pool(name="ppw", bufs=2, space="PSUM"))
                for g in range(KVH):
                    if fp16:
                        wkh = wkp.tile([P, KCH, P], DT, tag="wk", name=f"wkh{g}")
                        for q2 in range(2):
                            k2 = slice(q2 * (KCH // 2), (q2 + 1) * (KCH // 2))
                            nc.sync.dma_start(
                                wkh[:, k2, :],
                                t["wk_f"].ap()[g][:, k2, :])
                    else:
                        wkh = wkp.tile([P, KCH, P], BF16, tag="wk", name=f"wkh{g}")
                        wkl = wkp.tile([P, KCH, P], BF16, tag="wk", name=f"wkl{g}")
                        for q2 in range(2):
                            k2 = slice(q2 * (KCH // 2), (q2 + 1) * (KCH // 2))
                            nc.sync.dma_start(
                                wkh[:, k2, :],
                                t["wk_hi"].ap()[g][:, k2, :])
                            nc.sync.dma_start(
                                wkl[:, k2, :],
                                t["wk_lo"].ap()[g][:, k2, :])
                    ps = ppk.tile([P, LR], F32, tag="pk", name=f"pk{g}")
                    for ck in range(KCH):
                        if fp16:
                            nc.tensor.matmul(ps[:], wkh[:, ck, :], xh[:, ck, :],
                                             start=(ck == 0), stop=(ck == KCH - 1))
                        else:
                            nc.tensor.matmul(ps[:], wkh[:, ck, :], xh[:, ck, :],
                                             start=(ck == 0), stop=False)
                            nc.tensor.matmul(ps[:], wkh[:, ck, :], xl[:, ck, :],
                                             start=False, stop=False)
                            nc.tensor.matmul(ps[:], wkl[:, ck, :], xh[:, ck, :],
                                             start=False, stop=(ck == KCH - 1))
                    khs = kev.tile([P, LR], DT, tag="khx", name=f"khx{g}")
                    kls = None if fp16 else kev.tile([P, LR], BF16, tag="klx",
                                                     name=f"klx{g}")
                    _rope_out(nc, kev, ppw, ps, perm_t, cosTu_t, sinTu_t,
                              [khs[:]] if fp16 else [khs[:], kls[:]],
                              uid=f"k{g}")
                    # pack rows sub*128+p; f32 cols [g*64,(g+1)*64) (+lo at 512)
                    dst = kv_loc[:, :].rearrange("(sub p) c -> p sub c", p=P)
                    src_h = khs[:].rearrange(
                        "p (sub c) -> p sub c", sub=LR // P).bitcast(F32)
                    nc.sync.dma_start(dst[:, :, g * 64:(g + 1) * 64], src_h)
                    if not fp16:
                        src_l = kls[:].rearrange(
                            "p (sub c) -> p sub c", sub=LR // P).bitcast(F32)
                        nc.sync.dma_start(
                            dst[:, :, 512 + g * 64:512 + (g + 1) * 64], src_l)

            # ---- v projection (natural layout) ----
            with ExitStack() as vctx:
                wvs = vctx.enter_context(tc.tile_pool(name="wvs", bufs=8))
                vev = vctx.enter_context(tc.tile_pool(name="vev", bufs=3))
                ppv = vctx.enter_context(
                    tc.tile_pool(name="ppv", bufs=4, space="PSUM"))
                for cg in range(KVC // 512):             # 2 col groups of 512
                    csl = slice(cg * 512, (cg + 1) * 512)
                    pv = [ppv.tile([P, 512], F32, tag="pv",
                                   name=f"pv{cg}_{i}") for i in range(4)]
                    for ck in range(KCH):
                        wvt = wvs.tile([P, 512], DT, tag="wvt",
                                       name=f"wvt{cg}_{ck}")
                        nc.sync.dma_start(
                            wvt[:], t["wv_b"].ap()[ck * P:(ck + 1) * P, csl])
                        for rt in range(4):
                            rsl = slice(rt * P, (rt + 1) * P)
                            nc.tensor.matmul(pv[rt][:], xh[:, ck, rsl], wvt[:],
                                             start=(ck == 0),
                                             stop=(ck == KCH - 1))
                    for rt in range(4):
                        ve = vev.tile([P, 512], BF16, tag="ve",
                                      name=f"ve{cg}_{rt}")
                        nc.scalar.copy(ve[:], pv[rt][:])
                        nc.sync.dma_start(
                            kv_loc[rt * P:(rt + 1) * P,
                                   VOFF // 2 + cg * 256:VOFF // 2 + (cg + 1) * 256],
                            ve[:].bitcast(F32))

            # ---- AllGather of packed K^T | V ----
            if TIMING_R != 0:
                nc.scalar.dma_start(kv_full[0:LR, :], kv_loc[:, :])
            else:
                nc.gpsimd.collective_compute(
                    "AllGather", mybir.AluOpType.bypass,
                    replica_groups=[list(range(N_CORES))],
                    ins=[kv_loc.opt()],
                    outs=[kv_full.opt()],
                )

            # ---- q projection + RoPE ----
            with ExitStack() as qctx:
                wqp = qctx.enter_context(tc.tile_pool(name="wqp", bufs=5))
                qev = qctx.enter_context(tc.tile_pool(name="qev", bufs=2))
                ppq = qctx.enter_context(
                    tc.tile_pool(name="ppq", bufs=2, space="PSUM"))
                ppw2 = qctx.enter_context(
                    tc.tile_pool(name="ppw2", bufs=2, space="PSUM"))
                for h in range(H):
                    if fp16:
                        wqh = wqp.tile([P, KCH, P], DT, tag="wq", name=f"wqh{h}")
                        for q2 in range(2):
                            k2 = slice(q2 * (KCH // 2), (q2 + 1) * (KCH // 2))
                            nc.sync.dma_start(
                                wqh[:, k2, :],
                                t["wq_f"].ap()[h][:, k2, :])
                    else:
                        wqh = wqp.tile([P, KCH, P], BF16, tag="wq", name=f"wqh{h}")
                        wql = wqp.tile([P, KCH, P], BF16, tag="wq", name=f"wql{h}")
                        for q2 in range(2):
                            k2 = slice(q2 * (KCH // 2), (q2 + 1) * (KCH // 2))
                            nc.sync.dma_start(
                                wqh[:, k2, :],
                                t["wq_hi"].ap()[h][:, k2, :])
                            nc.sync.dma_start(
                                wql[:, k2, :],
                                t["wq_lo"].ap()[h][:, k2, :])
                    ps = ppq.tile([P, LR], F32, tag="pq", name=f"pq{h}")
                    for ck in range(KCH):
                        if fp16:
                            nc.tensor.matmul(ps[:], wqh[:, ck, :], xh[:, ck, :],
                                             start=(ck == 0), stop=(ck == KCH - 1))
                        else:
                            nc.tensor.matmul(ps[:], wqh[:, ck, :], xh[:, ck, :],
                                             start=(ck == 0), stop=False)
                            nc.tensor.matmul(ps[:], wqh[:, ck, :], xl[:, ck, :],
                                             start=False, stop=False)
                            nc.tensor.matmul(ps[:], wql[:, ck, :], xh[:, ck, :],
                                             start=False, stop=(ck == KCH - 1))
                    _rope_out(nc, qev, ppw2, ps, perm_t, cosT_t, sinT_t,
                              [qh[:, h, :]] if fp16 else [qh[:, h, :], ql[:, h, :]],
                              uid=f"q{h}")

        # ---------------- phase B: attention ----------------
        aoT_pool = ctx.enter_context(tc.tile_pool(name="aoT_pool", bufs=1))
        aoT = aoT_pool.tile([P, H, LR], BF16)

        _emit_attention(nc, tc, t, kv_full, qh, ql, aoT, ident_b,
                        bmask_t if causal else None, cb_t, causal, fp16,
                        KPACK, LOFF, VOFF)

        # ---------------- phase C: output projection ----------------
        with ExitStack() as cctx:
            wop = cctx.enter_context(tc.tile_pool(name="wop", bufs=2))
            osb = cctx.enter_context(tc.tile_pool(name="osb", bufs=3))
            ps_o = cctx.enter_context(
                tc.tile_pool(name="ps_o", bufs=4, space="PSUM"))
            for dg in range(D // 512):  # 8
                dsl = slice(dg * 512, (dg + 1) * 512)
                wot = wop.tile([P, KCH, 512], BF16, tag="wo", name=f"wo{dg}")
                wo_src = t["wo_b"].ap()[dg]
                for q4 in range(4):
                    ksl4 = slice(q4 * (KCH // 4), (q4 + 1) * (KCH // 4))
                    nc.sync.dma_start(wot[:, ksl4, :], wo_src[:, ksl4, :])
                for rt in range(4):
                    rsl = slice(rt * P, (rt + 1) * P)
                    ps = ps_o.tile([P, 512], F32, tag="po",
                                   name=f"po{dg}_{rt}")
                    for ck in range(KCH):
                        nc.tensor.matmul(ps[:], aoT[:, ck, rsl], wot[:, ck, :],
                                         start=(ck == 0), stop=(ck == KCH - 1))
                    ot = osb.tile([P, 512], F32, tag="ot", name=f"ot{dg}_{rt}")
                    nc.scalar.copy(ot[:], ps[:])
                    nc.sync.dma_start(out_ext.ap()[rt * P:(rt + 1) * P, dsl], ot[:])


def _emit_attention(nc, tc, t, kv_full, qh, ql, aoT, ident_b, bmask_t,
                    cb_t, causal, fp16, KPACK, LOFF, VOFF):
    from contextlib import ExitStack
    DT = F16 if fp16 else BF16

    with ExitStack() as bctx:
        kst = bctx.enter_context(tc.tile_pool(name="kst", bufs=3))
        vst = bctx.enter_context(tc.tile_pool(name="vst", bufs=3))
        scp = bctx.enter_context(tc.tile_pool(name="scp", bufs=3))
        atp = bctx.enter_context(tc.tile_pool(name="atp", bufs=2))
        sml = bctx.enter_context(tc.tile_pool(name="sml", bufs=8))
        ps_sc = bctx.enter_context(
            tc.tile_pool(name="ps_sc", bufs=2, space="PSUM"))
        ps_tr = bctx.enter_context(
            tc.tile_pool(name="ps_tr", bufs=2, space="PSUM"))
        ps_av = bctx.enter_context(
            tc.tile_pool(name="ps_av", bufs=2, space="PSUM"))

        kvb = kv_full[:, :].bitcast(DT)
        src = kvb.rearrange(
            "(r e hj p) c -> p r e hj c", p=P, e=B, hj=SL // P)
        srcv = kv_full[:, :].bitcast(BF16).rearrange(
            "(r e hj p) c -> p r e hj c", p=P, e=B, hj=SL // P)

        def softmax_group(psc, eraw_sl, nmax_ap, rsum_ap, masked, uid):
            """psc [P,1024] raw scores -> eraw = exp(psc + bias); accumulates
            rsum.  fp16 mode: constant bias (scores are small), masked via 0/1
            multiply; hp mode: additive -1e9 mask, per-group row max."""
            if masked and not fp16:
                pass  # multiplicative mask below
            if not fp16:
                if masked:
                    nc.vector.tensor_add(psc[:], psc[:], bmask_t[:])
                nc.vector.tensor_reduce(
                    nmax_ap, psc[:], axis=mybir.AxisListType.XY,
                    op=mybir.AluOpType.max, negate=True)
                nc.scalar.activation(
                    eraw_sl, psc[:], mybir.ActivationFunctionType.Exp,
                    bias=nmax_ap, scale=1.0, accum_out=rsum_ap)
                return
            if masked:
                nc.scalar.activation(
                    eraw_sl, psc[:], mybir.ActivationFunctionType.Exp,
                    bias=cb_t[:], scale=1.0)
                nc.vector.scalar_tensor_tensor(
                    out=eraw_sl, in0=eraw_sl, scalar=1.0, in1=bmask_t[:],
                    op0=mybir.AluOpType.bypass, op1=mybir.AluOpType.mult,
                    accum_out=rsum_ap)
            else:
                nc.scalar.activation(
                    eraw_sl, psc[:], mybir.ActivationFunctionType.Exp,
                    bias=cb_t[:], scale=1.0, accum_out=rsum_ap)

        pend = []

        def flush_pend():
            for fn in pend:
                fn()
            pend.clear()

        for b in range(B):
            for g in range(KVH):
                # stage K^T (+lo) and V, band-major chunk order (hj*8+r)
                kh_s = kst.tile([P, NKT, P], DT, tag="khs", name=f"khs{b}_{g}")
                kl_s = None if fp16 else kst.tile([P, NKT, P], BF16, tag="kls",
                                                  name=f"kls{b}_{g}")
                vn = vst.tile([P, NKT, HD], BF16, tag="vn",
                              name=f"vn{b}_{g}")
                kh_v = kh_s[:].rearrange("p (hj r) c -> p hj r c", hj=SL // P)
                vn_v = vn[:].rearrange("p (hj r) c -> p hj r c", hj=SL // P)
                for hj in range(SL // P):
                    nc.sync.dma_start(
                        kh_v[:, hj, :, :],
                        src[:, :, b, hj, g * P:(g + 1) * P])
                    nc.sync.dma_start(
                        vn_v[:, hj, :, :],
                        srcv[:, :, b, hj, VOFF + g * P:VOFF + (g + 1) * P])
                    if not fp16:
                        kl_v = kl_s[:].rearrange("p (hj r) c -> p hj r c",
                                                 hj=SL // P)
                        nc.sync.dma_start(
                            kl_v[:, hj, :, :],
                            src[:, :, b, hj, LOFF + g * P:LOFF + (g + 1) * P])
                kh_m = kh_s[:].rearrange("p a c -> p (a c)")
                kl_m = None if fp16 else kl_s[:].rearrange("p a c -> p (a c)")

                for r in range(NREP):
                    h = g * NREP + r
                    u0 = f"{b}_{g}_{r}"
                    aT = atp.tile([P, NKT, 2 * P], BF16, tag="aT",
                                  name=f"aT{u0}")
                    eraw0 = scp.tile([P, 2048], BF16, tag="er0",
                                     name=f"er0{u0}")
                    eraw1 = scp.tile([P, 2048], BF16, tag="er1",
                                     name=f"er1{u0}")

                    def qk_matmuls(psc, qrsl, cols, h=h, kh_m=kh_m, kl_m=kl_m):
                        # cols: slice of kh_m DT columns (multiple of 512)
                        n512 = (cols.stop - cols.start) // 512
                        for kt in range(n512):
                            ksl = slice(cols.start + kt * 512,
                                        cols.start + (kt + 1) * 512)
                            osl = slice(kt * 512, (kt + 1) * 512)
                            if fp16:
                                nc.tensor.matmul(
                                    psc[:, osl], qh[:, h, qrsl], kh_m[:, ksl],
                                    start=True, stop=True)
                            else:
                                nc.tensor.matmul(
                                    psc[:, osl], qh[:, h, qrsl], kh_m[:, ksl],
                                    start=True, stop=False)
                                nc.tensor.matmul(
                                    psc[:, osl], ql[:, h, qrsl], kh_m[:, ksl],
                                    start=False, stop=False)
                                nc.tensor.matmul(
                                    psc[:, osl], qh[:, h, qrsl], kl_m[:, ksl],
                                    start=False, stop=True)

                    def softmax_tile(qrsl, eraw, halves, uid,
                                     qk_matmuls=qk_matmuls):
                        """halves: list of (col_slice, masked).  Writes
                        UNNORMALIZED exp into eraw; returns per-half diag
                        normalizer matrices (folded into the PE transpose)."""
                        nh = len(halves)
                        rowmax = sml.tile([P, nh], F32, tag="rmax",
                                          name=f"rm{uid}")
                        rsum = sml.tile([P, nh], F32, tag="rsum",
                                        name=f"rs{uid}")
                        for i, (csl, masked) in enumerate(halves):
                            psc = ps_sc.tile([P, 1024], F32, tag="psc",
                                             name=f"psc{uid}_{i}")
                            qk_matmuls(psc, qrsl, csl)
                            softmax_group(
                                psc, eraw[:, csl.start:csl.stop],
                                rowmax[:, i:i + 1], rsum[:, i:i + 1],
                                masked, uid=f"{uid}_{i}")
                        recip = sml.tile([P, 1], F32, tag="recip",
                                         name=f"rc{uid}")
                        if fp16 or nh == 1:
                            # common exp bias across halves -> plain sum
                            if nh == 1:
                                nc.vector.reciprocal(recip[:], rsum[:])
                            else:
                                tots = sml.tile([P, 1], F32, tag="tots",
                                                name=f"to{uid}")
                                nc.vector.tensor_reduce(
                                    tots[:], rsum[:],
                                    axis=mybir.AxisListType.XY,
                                    op=mybir.AluOpType.add)
                                nc.vector.reciprocal(recip[:], tots[:])
                            wd = sml.tile([P, P], BF16, tag="wd",
                                          name=f"wd{uid}")
                            nc.vector.tensor_scalar_mul(wd[:], ident_b[:],
                                                        recip[:])
                            return [wd] * nh
                        # hp: per-half max -> fixup scales th_h/Z
                        negmax = sml.tile([P, 1], F32, tag="nmax",
                                          name=f"nx{uid}")
                        nc.vector.tensor_reduce(
                            negmax[:], rowmax[:], axis=mybir.AxisListType.XY,
                            op=mybir.AluOpType.min)
                        th = sml.tile([P, nh], F32, tag="th", name=f"th{uid}")
                        for i in range(nh):
                            nc.scalar.activation(
                                th[:, i:i + 1], rowmax[:, i:i + 1],
                                mybir.ActivationFunctionType.Exp,
                                bias=negmax[:], scale=-1.0)
                        prod = sml.tile([P, nh], F32, tag="prod",
                                        name=f"pr{uid}")
                        tots = sml.tile([P, 1], F32, tag="tots",
                                        name=f"to{uid}")
                        nc.vector.tensor_mul(prod[:], rsum[:], th[:])
                        nc.vector.tensor_reduce(
                            tots[:], prod[:], axis=mybir.AxisListType.XY,
                            op=mybir.AluOpType.add)
                        nc.vector.reciprocal(recip[:], tots[:])
                        sc2 = sml.tile([P, nh], F32, tag="sc2",
                                       name=f"sc{uid}")
                        nc.vector.tensor_scalar_mul(sc2[:], th[:], recip[:])
                        wds = []
                        for i in range(nh):
                            wd = sml.tile([P, P], BF16, tag="wd",
                                          name=f"wd{uid}_{i}")
                            nc.vector.tensor_scalar_mul(wd[:], ident_b[:],
                                                        sc2[:, i:i + 1])
                            wds.append(wd)
                        return wds

                    q0sl = slice(b * SL, b * SL + P)
                    q1sl = slice(b * SL + P, b * SL + 2 * P)
                    if causal:
                        halves0 = [(slice(0, 1024), True)]
                        halves1 = [(slice(0, 1024), False),
                                   (slice(1024, 2048), True)]
                        nj0 = 8
                    else:
                        halves0 = [(slice(0, 1024), False),
                                   (slice(1024, 2048), False)]
                        halves1 = halves0
                        nj0 = NKT
                    wds0 = softmax_tile(q0sl, eraw0, halves0, f"a{u0}")
                    wds1 = softmax_tile(q1sl, eraw1, halves1, f"b{u0}")

                    def emit_back(eraw0=eraw0, eraw1=eraw1, wds0=wds0,
                                  wds1=wds1, aT=aT, vn=vn, nj0=nj0, h=h,
                                  b=b, u0=u0):
                        # transpose attn -> aT (normalization folded into the
                        # transpose weight: diag(scale) instead of identity)
                        tb_i = 0
                        for src_er, nj, tcol, wds in ((eraw0, nj0, 0, wds0),
                                                      (eraw1, NKT, 1, wds1)):
                            for tb in range(nj // 4):
                                pst = ps_tr.tile([P, 512], F32, tag="ptr",
                                                 name=f"ptr{u0}_{tcol}_{tb}")
                                for jj in range(4):
                                    j = tb * 4 + jj
                                    wd = wds[min(j // 8, len(wds) - 1)]
                                    nc.tensor.matmul(
                                        pst[:, jj * P:(jj + 1) * P],
                                        src_er[:, j * P:(j + 1) * P],
                                        wd[:], start=True, stop=True)
                                dst = aT[:, tb * 4:(tb + 1) * 4,
                                         tcol * P:(tcol + 1) * P]
                                srcp = pst[:].rearrange("p (a c) -> p a c",
                                                        a=4)
                                if tb_i % 2 == 0:
                                    nc.vector.tensor_copy(dst, srcp)
                                else:
                                    nc.scalar.copy(dst, srcp)
                                tb_i += 1

                        # attn @ V
                        povw = 384 if causal else 256
                        pov = ps_av.tile([P, povw], F32, tag="pov",
                                         name=f"pov{u0}")
                        pov01 = pov[:, 0:256]
                        for j in range(nj0):
                            nc.tensor.matmul(pov01, vn[:, j, :], aT[:, j, :],
                                             start=(j == 0),
                                             stop=(j == nj0 - 1))
                        if causal:
                            pov1b = pov[:, 256:384]
                            for j in range(8, 16):
                                nc.tensor.matmul(pov1b, vn[:, j, :],
                                                 aT[:, j, P:2 * P],
                                                 start=(j == 8),
                                                 stop=(j == 15))
                            nc.scalar.copy(aoT[:, h, b * SL:b * SL + P],
                                           pov[:, 0:P])
                            p1tmp = sml.tile([P, P], F32, tag="p1t",
                                             name=f"p1t{u0}")
                            nc.scalar.copy(p1tmp[:], pov[:, P:2 * P])
                            nc.vector.tensor_add(
                                aoT[:, h, b * SL + P:b * SL + 2 * P],
                                pov1b, p1tmp[:])
                        else:
                            nc.scalar.copy(aoT[:, h, b * SL:(b + 1) * SL],
                                           pov01)

                    flush_pend()
                    pend.append(emit_back)
        flush_pend()


def _emit_emask(nc, tc, t):
    """Legacy path: arbitrary mask via exp(mask) multiply, full attention,
    split-bf16 precision.  (Structure of the original baseline kernel.)"""
    from contextlib import ExitStack
    from concourse.masks import make_identity
    out_ext = t["out_ext"]
    PACK = 1536
    with ExitStack() as ctx:
        const = ctx.enter_context(tc.tile_pool(name="const", bufs=1))
        qsp_pool = ctx.enter_context(tc.tile_pool(name="qsp_pool", bufs=1))
        dram = ctx.enter_context(tc.tile_pool(name="dram", bufs=1, space="DRAM"))

        kv_loc = dram.tile([LR, PACK], F32)
        kv_full = dram.tile([N_CORES * LR, PACK], F32, addr_space="Shared")

        ident_b = const.tile([P, P], BF16)
        make_identity(nc, ident_b)

        qh = qsp_pool.tile([P, H, LR], BF16)
        ql = qsp_pool.tile([P, H, LR], BF16)

        with ExitStack() as actx:
            ac = actx.enter_context(tc.tile_pool(name="ac", bufs=1))
            xt_pool = actx.enter_context(tc.tile_pool(name="xt_pool", bufs=1))

            cosT_t = ac.tile([P, SL], F32)
            sinT_t = ac.tile([P, SL], F32)
            cosTu_t = ac.tile([P, SL], F32)
            sinTu_t = ac.tile([P, SL], F32)
            nc.sync.dma_start(cosT_t[:], t["cosT"].ap()[:, :])
            nc.sync.dma_start(sinT_t[:], t["sinT"].ap()[:, :])
            nc.sync.dma_start(cosTu_t[:], t["cosTu"].ap()[:, :])
            nc.sync.dma_start(sinTu_t[:], t["sinTu"].ap()[:, :])
            perm_t = ac.tile([P, P], F32)
            nc.sync.dma_start(perm_t[:], t["perm"].ap()[:, :])

            xh = xt_pool.tile([P, KCH, LR], BF16)
            xl = xt_pool.tile([P, KCH, LR], BF16)
            xh_src = t["xt_hi"].ap()
            xl_src = t["xt_lo"].ap()
            for q4 in range(4):
                ksl4 = slice(q4 * (KCH // 4), (q4 + 1) * (KCH // 4))
                nc.sync.dma_start(xh[:, ksl4, :], xh_src[:, ksl4, :])
                nc.sync.dma_start(xl[:, ksl4, :], xl_src[:, ksl4, :])

            with ExitStack() as kctx:
                wkp = kctx.enter_context(tc.tile_pool(name="wkp", bufs=5))
                kev = kctx.enter_context(tc.tile_pool(name="kev", bufs=2))
                ppk = kctx.enter_context(
                    tc.tile_pool(name="ppk", bufs=2, space="PSUM"))
                ppw = kctx.enter_context(
                    tc.tile_pool(name="ppw", bufs=2, space="PSUM"))
                for g in range(KVH):
                    wkh = wkp.tile([P, KCH, P], BF16, tag="wk", name=f"wkh{g}")
                    wkl = wkp.tile([P, KCH, P], BF16, tag="wk", name=f"wkl{g}")
                    for q2 in range(2):
                        k2 = slice(q2 * (KCH // 2), (q2 + 1) * (KCH // 2))
                        nc.sync.dma_start(
                            wkh[:, k2, :],
                            t["wk_hi"].ap()[g][:, k2, :])
                        nc.sync.dma_start(
                            wkl[:, k2, :],
                            t["wk_lo"].ap()[g][:, k2, :])
                    ps = ppk.tile([P, LR], F32, tag="pk", name=f"pk{g}")
                    for ck in range(KCH):
                        nc.tensor.matmul(ps[:], wkh[:, ck, :], xh[:, ck, :],
                                         start=(ck == 0), stop=False)
                        nc.tensor.matmul(ps[:], wkh[:, ck, :], xl[:, ck, :],
                                         start=False, stop=False)
                        nc.tensor.matmul(ps[:], wkl[:, ck, :], xh[:, ck, :],
                                         start=False, stop=(ck == KCH - 1))
                    khs = kev.tile([P, LR], BF16, tag="khx", name=f"khx{g}")
                    kls = kev.tile([P, LR], BF16, tag="klx", name=f"klx{g}")
                    _rope_out(nc, kev, ppw, ps, perm_t, cosTu_t, sinTu_t,
                              [khs[:], kls[:]], uid=f"k{g}")
                    src_h = khs[:].rearrange(
                        "p (sub c) -> p sub c", sub=LR // P).bitcast(F32)
                    src_l = kls[:].rearrange(
                        "p (sub c) -> p sub c", sub=LR // P).bitcast(F32)
                    dst = kv_loc[:, :].rearrange("(sub p) c -> p sub c", p=P)
                    nc.sync.dma_start(dst[:, :, g * 64:(g + 1) * 64], src_h)
                    nc.sync.dma_start(
                        dst[:, :, 512 + g * 64:512 + (g + 1) * 64], src_l)

            with ExitStack() as vctx:
                wvs = vctx.enter_context(tc.tile_pool(name="wvs", bufs=8))
                vev = vctx.enter_context(tc.tile_pool(name="vev", bufs=3))
                ppv = vctx.enter_context(
                    tc.tile_pool(name="ppv", bufs=4, space="PSUM"))
                for cg in range(KVC // 512):
                    csl = slice(cg * 512, (cg + 1) * 512)
                    pv = [ppv.tile([P, 512], F32, tag="pv",
                                   name=f"pv{cg}_{i}") for i in range(4)]
                    for ck in range(KCH):
                        wvt = wvs.tile([P, 512], BF16, tag="wvt",
                                       name=f"wvt{cg}_{ck}")
                        nc.sync.dma_start(
                            wvt[:], t["wv_b"].ap()[ck * P:(ck + 1) * P, csl])
                        for rt in range(4):
                            rsl = slice(rt * P, (rt + 1) * P)
                            nc.tensor.matmul(pv[rt][:], xh[:, ck, rsl], wvt[:],
                                             start=(ck == 0),
                                             stop=(ck == KCH - 1))
                    for rt in range(4):
                        ve = vev.tile([P, 512], BF16, tag="ve",
                                      name=f"ve{cg}_{rt}")
                        nc.scalar.copy(ve[:], pv[rt][:])
                        nc.sync.dma_start(
                            kv_loc[rt * P:(rt + 1) * P,
                                   1024 + cg * 256:1024 + (cg + 1) * 256],
                            ve[:].bitcast(F32))

            if TIMING_R != 0:
                nc.scalar.dma_start(kv_full[0:LR, :], kv_loc[:, :])
            else:
                nc.gpsimd.collective_compute(
                    "AllGather", mybir.AluOpType.bypass,
                    replica_groups=[list(range(N_CORES))],
                    ins=[kv_loc.opt()],
                    outs=[kv_full.opt()],
                )

            with ExitStack() as qctx:
                wqp = qctx.enter_context(tc.tile_pool(name="wqp", bufs=5))
                qev = qctx.enter_context(tc.tile_pool(name="qev", bufs=2))
                ppq = qctx.enter_context(
                    tc.tile_pool(name="ppq", bufs=2, space="PSUM"))
                ppw2 = qctx.enter_context(
                    tc.tile_pool(name="ppw2", bufs=2, space="PSUM"))
                for h in range(H):
                    wqh = wqp.tile([P, KCH, P], BF16, tag="wq", name=f"wqh{h}")
                    wql = wqp.tile([P, KCH, P], BF16, tag="wq", name=f"wql{h}")
                    for q2 in range(2):
                        k2 = slice(q2 * (KCH // 2), (q2 + 1) * (KCH // 2))
                        nc.sync.dma_start(
                            wqh[:, k2, :],
                            t["wq_hi"].ap()[h][:, k2, :])
                        nc.sync.dma_start(
                            wql[:, k2, :],
                            t["wq_lo"].ap()[h][:, k2, :])
                    ps = ppq.tile([P, LR], F32, tag="pq", name=f"pq{h}")
                    for ck in range(KCH):
                        nc.tensor.matmul(ps[:], wqh[:, ck, :], xh[:, ck, :],
                                         start=(ck == 0), stop=False)
                        nc.tensor.matmul(ps[:], wqh[:, ck, :], xl[:, ck, :],
                                         start=False, stop=False)
                        nc.tensor.matmul(ps[:], wql[:, ck, :], xh[:, ck, :],
                                         start=False, stop=(ck == KCH - 1))
                    _rope_out(nc, qev, ppw2, ps, perm_t, cosT_t, sinT_t,
                              [qh[:, h, :], ql[:, h, :]], uid=f"q{h}")

        aoT_pool = ctx.enter_context(tc.tile_pool(name="aoT_pool", bufs=1))
        aoT = aoT_pool.tile([P, H, LR], BF16)

        with ExitStack() as bctx:
            bc = bctx.enter_context(tc.tile_pool(name="bc", bufs=1))
            kst = bctx.enter_context(tc.tile_pool(name="kst", bufs=3))
            vst = bctx.enter_context(tc.tile_pool(name="vst", bufs=3))
            scp = bctx.enter_context(tc.tile_pool(name="scp", bufs=3))
            atp = bctx.enter_context(tc.tile_pool(name="atp", bufs=3))
            sml = bctx.enter_context(tc.tile_pool(name="sml", bufs=8))
            ps_sc = bctx.enter_context(
                tc.tile_pool(name="ps_sc", bufs=2, space="PSUM"))
            ps_tr = bctx.enter_context(
                tc.tile_pool(name="ps_tr", bufs=2, space="PSUM"))
            ps_av = bctx.enter_context(
                tc.tile_pool(name="ps_av", bufs=2, space="PSUM"))

            mask_t = bc.tile([P, 2, S], F32)
            nc.sync.dma_start(
                mask_t[:], t["emask"].ap().rearrange("(a p) c -> p a c", p=P))

            kvb = kv_full[:, :].bitcast(BF16)
            src = kvb.rearrange(
                "(r e hj p) c -> p r e hj c", p=P, e=B, hj=SL // P)
            for b in range(B):
                for g in range(KVH):
                    kh_s = kst.tile([P, NKT, P], BF16, tag="khs",
                                    name=f"khs{b}_{g}")
                    kl_s = kst.tile([P, NKT, P], BF16, tag="kls",
                                    name=f"kls{b}_{g}")
                    vn = vst.tile([P, NKT, HD], BF16, tag="vn",
                                  name=f"vn{b}_{g}")
                    for hj in range(SL // P):
                        kh_v = kh_s[:].rearrange("p (r hj) c -> p r hj c",
                                                 hj=SL // P)
                        kl_v = kl_s[:].rearrange("p (r hj) c -> p r hj c",
                                                 hj=SL // P)
                        vnv = vn[:].rearrange("p (r hj) c -> p r hj c",
                                              hj=SL // P)
                        nc.sync.dma_start(
                            kh_v[:, :, hj, :],
                            src[:, :, b, hj, g * P:(g + 1) * P])
                        nc.sync.dma_start(
                            kl_v[:, :, hj, :],
                            src[:, :, b, hj, 1024 + g * P:1024 + (g + 1) * P])
                        nc.sync.dma_start(
                            vnv[:, :, hj, :],
                            src[:, :, b, hj, 2048 + g * P:2048 + (g + 1) * P])
                    kh_m = kh_s[:].rearrange("p a c -> p (a c)")
                    kl_m = kl_s[:].rearrange("p a c -> p (a c)")

                    for r in range(NREP):
                        h = g * NREP + r
                        aT = atp.tile([P, NKT, SL], BF16, tag="aT",
                                      name=f"aT{b}_{g}_{r}")
                        for qs in range(SL // P):
                            u = f"{b}_{g}_{r}_{qs}"
                            qrsl = slice(b * SL + qs * P,
                                         b * SL + (qs + 1) * P)
                            eraw = scp.tile([P, S], BF16, tag="eraw",
                                            name=f"eraw{u}")
                            rowmax = sml.tile([P, 2], F32, tag="rmax",
                                              name=f"rmax{u}")
                            rsum = sml.tile([P, 2], F32, tag="rsum",
                                            name=f"rsum{u}")
                            negmax = sml.tile([P, 1], F32, tag="nmax",
                                              name=f"nmax{u}")
                            for half in range(2):
                                psc = ps_sc.tile([P, 1024], F32, tag="psc",
                                                 name=f"psc{u}_{half}")
                                for kt in range(2):
                                    ksl = slice((half * 2 + kt) * 512,
                                                (half * 2 + kt + 1) * 512)
                                    osl = slice(kt * 512, (kt + 1) * 512)
                                    nc.tensor.matmul(
                                        psc[:, osl], qh[:, h, qrsl],
                                        kh_m[:, ksl], start=True, stop=False)
                                    nc.tensor.matmul(
                                        psc[:, osl], ql[:, h, qrsl],
                                        kh_m[:, ksl], start=False, stop=False)
                                    nc.tensor.matmul(
                                        psc[:, osl], qh[:, h, qrsl],
                                        kl_m[:, ksl], start=False, stop=True)
                                nc.vector.tensor_reduce(
                                    rowmax[:, half:half + 1], psc[:],
                                    axis=mybir.AxisListType.XY,
                                    op=mybir.AluOpType.max, negate=True)
                                nc.scalar.activation(
                                    eraw[:, half * 1024:(half + 1) * 1024],
                                    psc[:],
                                    mybir.ActivationFunctionType.Exp,
                                    bias=rowmax[:, half:half + 1], scale=1.0,
                                    accum_out=rsum[:, half:half + 1])
                            nc.vector.tensor_reduce(
                                negmax[:], rowmax[:],
                                axis=mybir.AxisListType.XY,
                                op=mybir.AluOpType.min)
                            th = sml.tile([P, 2], F32, tag="th", name=f"th{u}")
                            for half in range(2):
                                nc.scalar.activation(
                                    th[:, half:half + 1],
                                    rowmax[:, half:half + 1],
                                    mybir.ActivationFunctionType.Exp,
                                    bias=negmax[:], scale=-1.0)
                            tots = sml.tile([P, 1], F32, tag="tots",
                                            name=f"tots{u}")
                            prod = sml.tile([P, 2], F32, tag="prod",
                                            name=f"prod{u}")
                            nc.vector.tensor_mul(prod[:], rsum[:], th[:])
                            nc.vector.scalar_tensor_tensor(
                                out=eraw[:], in0=eraw[:], scalar=1.0,
                                in1=mask_t[:, qs, :],
                                op0=mybir.AluOpType.bypass,
                                op1=mybir.AluOpType.mult)
                            for half in range(2):
                                hsl = slice(half * 1024, (half + 1) * 1024)
                                nc.vector.tensor_reduce(
                                    rsum[:, half:half + 1], eraw[:, hsl],
                                    axis=mybir.AxisListType.XY,
                                    op=mybir.AluOpType.add)
                            nc.vector.tensor_mul(prod[:], rsum[:], th[:])
                            nc.vector.tensor_reduce(
                                tots[:], prod[:],
                                axis=mybir.AxisListType.XY,
                                op=mybir.AluOpType.add)
                            recip = sml.tile([P, 1], F32, tag="recip",
                                             name=f"recip{u}")
                            nc.vector.reciprocal(recip[:], tots[:])
                            sc2 = sml.tile([P, 2], F32, tag="sc2",
                                           name=f"sc2{u}")
                            nc.vector.tensor_scalar_mul(sc2[:], th[:],
                                                        recip[:])
                            for half in range(2):
                                hsl = slice(half * 1024, (half + 1) * 1024)
                                nc.vector.tensor_scalar_mul(
                                    eraw[:, hsl], eraw[:, hsl],
                                    sc2[:, half:half + 1])
                            for tb in range(4):
                                pst = ps_tr.tile([P, 512], BF16, tag="ptr",
                                                 name=f"ptr{u}_{tb}")
                                for j in range(4):
                                    jj = tb * 4 + j
                                    nc.tensor.transpose(
                                        pst[:, j * P:(j + 1) * P],
                                        eraw[:, jj * P:(jj + 1) * P],
                                        ident_b)
                                dst = aT[:, tb * 4:(tb + 1) * 4,
                                         qs * P:(qs + 1) * P]
                                srcp = pst[:].rearrange(
                                    "p (a c) -> p a c", a=4)
                                if (tb + qs) % 2 == 0:
                                    nc.vector.tensor_copy(dst, srcp)
                                else:
                                    nc.scalar.copy(dst, srcp)
                        pov = ps_av.tile([P, SL], F32, tag="pov",
                                         name=f"pov{b}_{g}_{r}")
                        for j in range(NKT):
                            nc.tensor.matmul(pov[:], vn[:, j, :], aT[:, j, :],
                                             start=(j == 0),
                                             stop=(j == NKT - 1))
                        nc.scalar.copy(aoT[:, h, b * SL:(b + 1) * SL], pov[:])

        with ExitStack() as cctx:
            wop = cctx.enter_context(tc.tile_pool(name="wop", bufs=2))
            osb = cctx.enter_context(tc.tile_pool(name="osb", bufs=3))
            ps_o = cctx.enter_context(
                tc.tile_pool(name="ps_o", bufs=4, space="PSUM"))
            for dg in range(D // 512):
                dsl = slice(dg * 512, (dg + 1) * 512)
                wot = wop.tile([P, KCH, 512], BF16, tag="wo", name=f"wo{dg}")
                wo_src = t["wo_b"].ap()[dg]
                for q4 in range(4):
                    ksl4 = slice(q4 * (KCH // 4), (q4 + 1) * (KCH // 4))
                    nc.sync.dma_start(wot[:, ksl4, :], wo_src[:, ksl4, :])
                for rt in range(4):
                    rsl = slice(rt * P, (rt + 1) * P)
                    ps = ps_o.tile([P, 512], F32, tag="po",
                                   name=f"po{dg}_{rt}")
                    for ck in range(KCH):
                        nc.tensor.matmul(ps[:], aoT[:, ck, rsl], wot[:, ck, :],
                                         start=(ck == 0), stop=(ck == KCH - 1))
                    ot = osb.tile([P, 512], F32, tag="ot", name=f"ot{dg}_{rt}")
                    nc.scalar.copy(ot[:], ps[:])
                    nc.sync.dma_start(out_ext.ap()[rt * P:(rt + 1) * P, dsl],
                                      ot[:])


# --------------------------------------------------------------------------
# host side
# --------------------------------------------------------------------------

def _split_bf16(a):
    hi = a.astype(ml_dtypes.bfloat16)
    lo = (a - hi.astype(np.float32)).astype(ml_dtypes.bfloat16)
    return hi, lo


def _tile_w(w, nh):
    # [D, nh*HD] -> [nh, P, KCH, P] with [p, k, c] = w[k*P+p, h*HD+c]
    return np.ascontiguousarray(
        w.reshape(KCH, P, nh, P).transpose(2, 1, 0, 3))


def _tile_wo(wo):
    # [H*HD, D] -> [8, P, KCH, 512] with [dg, p, k, c] = wo[k*P+p, dg*512+c]
    return np.ascontiguousarray(
        wo.reshape(KCH, P, 8, 512).transpose(2, 1, 0, 3))


def _tile_x(xt):
    # [D, LR] -> [P, KCH, LR]
    return np.ascontiguousarray(xt.reshape(KCH, P, LR).transpose(1, 0, 2))


def _rows_slice(mode, c):
    # strided rows for the main paths, contiguous for the legacy path
    if mode == "emask":
        return slice(c * SL, (c + 1) * SL)
    return slice(c, S, N_CORES)


def _host_prep(mode, x, wq, wk, wv, wo, freqs_cos, freqs_sin, mask):
    fp16 = mode.endswith("16")
    causal = mode[0] == "c"
    scale = 1.0 / np.sqrt(HD)

    if fp16:
        wq_f = _tile_w(wq, H).astype(np.float16)
        wk_f = _tile_w(wk, KVH).astype(np.float16)
        wv_b = wv.astype(np.float16)
        wo_b = _tile_wo(wo.astype(ml_dtypes.bfloat16))
        # constant exp bias: scores are bounded by ~6 sigma in this regime
        sx = float(x.std())
        sig = (sx * float(wq.std()) * math.sqrt(D)) * \
              (sx * float(wk.std()) * math.sqrt(D))
        cb = np.full((P, 1), -(6.0 * sig + 2.0), np.float32)
    else:
        wq_hi, wq_lo = (_tile_w(a, H) for a in _split_bf16(wq))
        wk_hi, wk_lo = (_tile_w(a, KVH) for a in _split_bf16(wk))
        wv_b = wv.astype(ml_dtypes.bfloat16)
        wo_b = _tile_wo(wo.astype(ml_dtypes.bfloat16))

    perm = np.zeros((P, P), np.float32)
    idx = np.arange(P)
    perm[idx, idx ^ 1] = 1.0  # pair swap

    in_maps = []
    for c in range(N_CORES):
        sl = _rows_slice(mode, c)
        x_loc = np.concatenate([x[0, sl], x[1, sl]], axis=0)  # [LR, D]
        xt = np.ascontiguousarray(x_loc.T)                    # [D, LR]

        fc = freqs_cos[sl]  # [SL, HD//2]
        fs = freqs_sin[sl]
        # transposed layout: freq i on partitions 2i/2i+1; sin sign: -s on
        # even rows, +s on odd rows.  q version carries the 1/sqrt(HD) scale.
        cosTu = np.repeat(fc.T, 2, axis=0)                    # [HD, SL]
        sinTu = np.repeat(fs.T, 2, axis=0).copy()
        sinTu[0::2] *= -1.0
        cosT = cosTu * scale
        sinT = sinTu * scale

        m = {
            "wv_b": wv_b, "wo_b": wo_b,
            "cosT": np.ascontiguousarray(cosT),
            "sinT": np.ascontiguousarray(sinT),
            "cosTu": np.ascontiguousarray(cosTu),
            "sinTu": np.ascontiguousarray(sinTu),
            "perm": perm,
        }
        if fp16:
            m["xt"] = _tile_x(xt.astype(np.float16))
            m["wq_f"] = wq_f
            m["wk_f"] = wk_f
            m["cbias"] = cb
        else:
            xt_hi, xt_lo = _split_bf16(xt)
            m["xt_hi"] = _tile_x(xt_hi)
            m["xt_lo"] = _tile_x(xt_lo)
            m["wq_hi"], m["wq_lo"] = wq_hi, wq_lo
            m["wk_hi"], m["wk_lo"] = wk_hi, wk_lo
        if causal:
            # band mask in staged coordinates: col rblk*128+ik <-> position
            # offset 8*ik+rblk within the band; row i <-> offset c+8*i.
            ik = np.arange(P)
            qoff = (c + 8 * ik)[:, None]                      # [128,1]
            rblk, kk = np.meshgrid(np.arange(8), np.arange(P), indexing="ij")
            koff = (8 * kk + rblk).reshape(-1)[None, :]       # [1,1024]
            allowed = koff <= qoff                            # [128,1024]
            if fp16:
                m["bmask"] = allowed.astype(ml_dtypes.bfloat16)
            else:
                m["bmask"] = np.where(allowed, 0.0, -1e9).astype(np.float32)
        if mode == "emask":
            mask_loc = np.exp(np.ascontiguousarray(
                np.broadcast_to(mask[0, 0], (S, S))[sl]))     # exp(mask)
            m["emask"] = mask_loc.astype(np.float32)
        in_maps.append(m)
    return in_maps


def _select_mode(x, wq, wk, wv, wo, mask):
    # causal / no-mask / general-mask
    if not np.any(mask != 0.0):
        masktype = "n"
    else:
        m2 = np.broadcast_to(mask[0, 0], (S, S))
        tril = np.tril(np.ones((S, S), bool))
        if np.all(m2[tril] == 0.0) and np.all(m2[~tril] <= -1e8):
            masktype = "c"
        else:
            return "emask"
    # precision: single-term fp16 suffices when softmax is diffuse
    sx = float(x.std())
    sq = sx * float(wq.std()) * math.sqrt(D)
    sk = sx * float(wk.std()) * math.sqrt(D)
    sig_score = sq * sk
    maxabs = max(float(np.abs(a).max()) for a in (x, wq, wk, wv, wo))
    fp16_ok = sig_score < 8.0 and maxabs < 2000.0 and sq < 500 and sk < 500
    return masktype + ("16" if fp16_ok else "hp")


def kernel(x, wq, wk, wv, wo, freqs_cos, freqs_sin, mask, start_pos=0, **_):
    x = np.asarray(x, dtype=np.float32)
    wq = np.asarray(wq, dtype=np.float32)
    wk = np.asarray(wk, dtype=np.float32)
    wv = np.asarray(wv, dtype=np.float32)
    wo = np.asarray(wo, dtype=np.float32)
    freqs_cos = np.asarray(freqs_cos, dtype=np.float32)
    freqs_sin = np.asarray(freqs_sin, dtype=np.float32)
    mask = np.asarray(mask, dtype=np.float32)

    mode = _select_mode(x, wq, wk, wv, wo, mask)
    if mode not in _GRAPH_CACHE:
        _GRAPH_CACHE[mode] = _build_graph(mode)
    nc = _GRAPH_CACHE[mode]

    in_maps = _host_prep(mode, x, wq, wk, wv, wo, freqs_cos, freqs_sin, mask)
    global _LAST_IN_MAPS
    _LAST_IN_MAPS = in_maps
    _GRAPH_CACHE["last_nc"] = nc
    _GRAPH_CACHE["last_mode"] = mode

    res = run_bass_kernel_spmd(nc, in_maps, core_ids=list(range(N_CORES)))

    out = np.empty((B, S, D), np.float32)
    for c in range(N_CORES):
        o = res.results[c]["out"]  # [LR, D]
        sl = _rows_slice(mode, c)
        out[0, sl] = o[:SL]
        out[1, sl] = o[SL:]
    return out


if __name__ == "__main__":
    rng = np.random.default_rng(0)
    inputs = {
        "x": rng.standard_normal((B, S, D), dtype=np.float32),
        "wq": rng.standard_normal((D, H * HD), dtype=np.float32) * 0.02,
        "wk": rng.standard_normal((D, KVC), dtype=np.float32) * 0.02,
        "wv": rng.standard_normal((D, KVC), dtype=np.float32) * 0.02,
        "wo": rng.standard_normal((H * HD, D), dtype=np.float32) * 0.02,
        "freqs_cos": rng.random((S, HD // 2), dtype=np.float32),
        "freqs_sin": rng.random((S, HD // 2), dtype=np.float32),
        "mask": np.zeros((1, 1, S, S), np.float32),
        "start_pos": 0,
    }
    out = kernel(**inputs)
    print("kernel output:", out.shape, out.dtype)
